# revision 25
# baseline (speedup 1.0000x reference)
"""AdaptiveGNN (GCN+GAT+SAGE mixture) on 8 Trainium2 NeuronCores — single
NEFF launch with on-device AllGather collectives.

Strategy (destination-sharded graph parallelism, one program):
 - Core k owns nodes [k*6250, (k+1)*6250); edges (plus self-loops) are
   sorted by destination on the host into a static per-window tile
   schedule shared by all 8 cores (window = 128 destination rows).
 - Phase A (local): GAT attention projections a1 = x @ [v_src|v_dst] and
   column sums of x. Writes [x | 1 | a_src] rows plus a trailing
   column-sum row into a per-core DRAM buffer.
 - AllGather #1 ([6273, 69] per core -> [50184, 69]): every core now has
   the full graph's source features + attention sources (halo exchange).
 - Gate MLP computed redundantly on every core from the 8 column-sum rows.
 - Phase B: layer 1 of all three branches for the local destination
   shard. Per edge-tile: indirect-DMA gather of source rows from the
   AllGathered buffer, local gather of a_dst rows, one-hot "selection"
   matmuls accumulate segment sums in PSUM. Window tails produce
   h1 = relu(bn(gcn1)), h2 = elu(gat1) @ W2 (+ attn scalars), hs = sage1;
   all written into the second per-core DRAM buffer [h1|h2|1|hs|a2src].
 - AllGather #2 ([6273, 194] per core -> [50184, 194]).
 - Phase C: layer 2 of all three branches + gated mix -> final rows.
 - Host sends only the x shard, edge streams and weights (~4MB/core)
   and receives the per-core output rows; one PJRT dispatch total.
"""

import hashlib
import os
import sys

sys.path.insert(0, "/opt/trn_rl_repo")

# Persistent XLA compilation cache: run_bass_via_pjrt re-jits a fresh
# closure every call, so jax's in-memory caches (weakref-keyed) can never
# hit; the disk cache is keyed on HLO bytes and skips the whole
# walrus+neuronxcc+load pipeline on repeat calls.
os.environ.setdefault("JAX_COMPILATION_CACHE_DIR",
                      os.path.expanduser("~/.cache/jax_bass_cache"))
os.environ.setdefault("JAX_PERSISTENT_CACHE_MIN_COMPILE_TIME_SECS", "0")
os.environ.setdefault("JAX_PERSISTENT_CACHE_MIN_ENTRY_SIZE_BYTES", "0")

import numpy as np

from concourse import bacc, bass, mybir, tile
from concourse import bass2jax as _b2j
from concourse.bass_utils import run_bass_kernel_spmd
import concourse.tile_sem_assignment as _tsa

# Memoize the bass_exec HLO -> NEFF-wrapped-HLO compile step. The jitted
# wrapper is rebuilt per call, so XLA recompiles the HLO each time; without
# this the BIR verify/walrus subprocess (~1.4s for this program) runs on
# every kernel() invocation. The serialized HloModuleProto differs across
# traces only in its module-id counter, so zero it for the cache key.
_HOOK_CACHE: dict = {}
_orig_neuronx_cc_hook = _b2j.neuronx_cc_hook


def _hook_key(code):
    import libneuronxla.proto.hlo_pb2 as _hlo_pb2
    p = _hlo_pb2.HloModuleProto.FromString(bytes(code))
    p.id = 0                           # per-trace module counter
    p.ClearField("stack_frame_index")  # caller source lines
    return hashlib.sha256(p.SerializeToString()).digest()


def _caching_neuronx_cc_hook(code, code_format, platform_version, file_prefix):
    if b"bass_exec" not in code:
        return _orig_neuronx_cc_hook(code, code_format, platform_version,
                                     file_prefix)
    try:
        key = _hook_key(code)
    except Exception:
        key = hashlib.sha256(bytes(code)).digest()
    r = _HOOK_CACHE.get(key)
    if r is None:
        r = _orig_neuronx_cc_hook(code, code_format, platform_version,
                                  file_prefix)
        _HOOK_CACHE[key] = r
    return r


_b2j.neuronx_cc_hook = _caching_neuronx_cc_hook

# Clamp Tile's DMA-completion semaphore lanes (kernel-tail Drain waits on
# every producer semaphore; walrus rejects instructions with too many
# sync waits).
_tsa.NUM_HWDGE_SEMS = 8
_tsa.NUM_SWDGE_GLOBAL_SEMS = 8

F32 = mybir.dt.float32
F16 = mybir.dt.float16
I32 = mybir.dt.int32
AF = mybir.ActivationFunctionType
ALU = mybir.AluOpType

NC_N = 8          # cores
D = 64            # feature dim
H1 = 4            # GAT hidden heads
WB = D + 1 + H1   # phase-B gather row: [x | 1 | a_src]            = 69
WC = 3 * D + 2    # phase-C gather row: [h1 | h2 | 1 | hs | a2src] = 194
NEG_SLOPE = 0.2
BN_EPS = 1e-5


# ----------------------------------------------------------------- host prep
def build_schedule(edge_index, n_nodes):
    """Sort edges (plus self-loops) by destination, shard by destination,
    and produce a tile schedule common to all cores plus per-core streams.
    Source indices are remapped into the AllGather row space
    (node n -> (n // shard) * (npad + 1) + n % shard)."""
    shard = n_nodes // NC_N
    nw = (shard + 127) // 128
    # per-core AllGather rows: +1 csum row, +1 pad so the collective's
    # element count stays even (NRT needs 8-byte-aligned collective sizes)
    nprow = nw * 128 + 2
    row = edge_index[0].astype(np.int64)
    col = edge_index[1].astype(np.int64)
    loops = np.arange(n_nodes, dtype=np.int64)
    r_all = np.concatenate([row, loops])
    c_all = np.concatenate([col, loops])

    # GCN symmetric normalization (self-loops included)
    deg = np.bincount(c_all, minlength=n_nodes).astype(np.float64)
    dis = np.where(deg > 0, deg ** -0.5, 0.0)
    wnorm_all = (dis[r_all] * dis[c_all]).astype(np.float32)
    # SAGE mean weights (real edges only; zero on appended self-loops)
    cnt = np.bincount(col, minlength=n_nodes).astype(np.float64)
    ws = (1.0 / np.maximum(cnt, 1.0))[col].astype(np.float32)
    wsage_all = np.concatenate([ws, np.zeros(n_nodes, np.float32)])
    # source node id -> AllGathered row
    rg_all = ((r_all // shard) * nprow + (r_all % shard)).astype(np.int64)

    per_core = []
    counts = np.zeros((NC_N, nw), dtype=np.int64)
    for k in range(NC_N):
        lo, hi = k * shard, (k + 1) * shard
        sel = np.nonzero((c_all >= lo) & (c_all < hi))[0]
        cl = c_all[sel] - lo
        order = np.argsort(cl, kind="stable")
        sel = sel[order]
        cl = cl[order]
        w_of = cl // 128
        cnts = np.bincount(w_of, minlength=nw)
        counts[k] = cnts
        per_core.append((sel, cl, cnts))

    tiles_w = np.maximum(1, (counts.max(axis=0) + 127) // 128)
    T = int(tiles_w.sum())

    streams = []
    for k in range(NC_N):
        sel, cl, cnts = per_core[k]
        idx_row = np.zeros(T * 128, np.int32)
        idx_colL = np.zeros(T * 128, np.int32)
        colrel = np.full(T * 128, -1.0, np.float32)
        wnorm = np.zeros(T * 128, np.float32)
        wsage = np.zeros(T * 128, np.float32)
        pos = 0      # position in padded stream
        epos = 0     # position in this core's sorted edge list
        for w in range(nw):
            cw = int(cnts[w])
            seg = sel[epos:epos + cw]
            base = pos
            idx_row[base:base + cw] = rg_all[seg]
            idx_colL[base:base + cw] = cl[epos:epos + cw]
            colrel[base:base + cw] = (cl[epos:epos + cw] % 128).astype(np.float32)
            wnorm[base:base + cw] = wnorm_all[seg]
            wsage[base:base + cw] = wsage_all[seg]
            epos += cw
            pos += int(tiles_w[w]) * 128
        st = {
            "idx_row": idx_row.reshape(T, 128).T.copy(),
            "idx_colL": idx_colL.reshape(T, 128).T.copy(),
            "colrel": colrel.reshape(T, 128).T.astype(np.float16),
            "wnorm": wnorm.reshape(T, 128).T.astype(np.float16),
            "wsage": wsage.reshape(T, 128).T.astype(np.float16),
        }
        streams.append(st)
    return streams, [int(t) for t in tiles_w], T, shard, nw


# ------------------------------------------------------------- device pieces
def _load_w(nc, pool, dram, shape, tag, in_dtype=None):
    ld = pool.tile(list(shape), in_dtype or F32, tag=tag + "_ld")
    nc.sync.dma_start(out=ld[:], in_=dram[:])
    t = pool.tile(list(shape), F32, tag=tag)
    nc.vector.tensor_copy(t[:], ld[:])
    return t


# ----------------------------------------------------------------- the build
def build_full(n_nodes, tiles_w, T):
    shard = n_nodes // NC_N
    nw = (shard + 127) // 128
    npad = nw * 128
    nprow = npad + 2   # +1 csum row, +1 pad row (8-byte collective align)
    gfull = NC_N * nprow
    rem = shard - (nw - 1) * 128       # rows in last output window

    nc = bacc.Bacc()
    dr = {}
    for nm, shp, dt in [
        ("xs65", [npad, D + 1], F16),
        ("idx_row", [128, T], I32), ("idx_colL", [128, T], I32),
        ("colrel", [128, T], F16), ("wnorm", [128, T], F16),
        ("wsage", [128, T], F16),
        ("vcat", [D, 2 * H1], F32),
        ("gw1", [D, D], F32), ("gb1", [1, D], F32),
        ("gw2", [D, 3], F32), ("gb2", [1, 3], F32),
        ("gcn_w1", [D, D], F32), ("gcn1_s", [D, 1], F32), ("gcn1_b", [D, 1], F32),
        ("sage_wl1", [D, D], F32), ("sage_wr1", [D, D], F32),
        ("sage_bl1", [D, 1], F32),
        ("w2A", [128, D], F32), ("w2B", [128, D], F32),
        ("v2u2", [128, 4], F32), ("w1h", [D, 4 * D], F32),
        ("b1c", [128, 2], F32),
        ("gcn_w2", [D, D], F32), ("gcn_b2c", [D, 1], F32),
        ("sage_wl2", [D, D], F32), ("sage_wr2", [D, D], F32),
        ("sage_bl2c", [D, 1], F32), ("gat_b2r", [1, D], F32),
    ]:
        dr[nm] = nc.dram_tensor(nm, shp, dt, kind="ExternalInput")
    out = nc.dram_tensor("out", [shard, D], F16, kind="ExternalOutput")
    c_ident = nc.inline_tensor(np.eye(128, dtype=np.float32), name="cident")
    c_iota = nc.inline_tensor(
        np.tile(np.arange(128, dtype=np.float32), (128, 1)), name="ciota")

    with tile.TileContext(nc) as tc:
        with (
            tc.tile_pool(name="const", bufs=1) as const,
            tc.tile_pool(name="wts", bufs=1) as wts,
            tc.tile_pool(name="stream", bufs=1) as stream,
            tc.tile_pool(name="stage", bufs=1) as stage,
            tc.tile_pool(name="dramp", bufs=1, space="DRAM") as dramp,
        ):
            ident = _load_w(nc, const, c_ident, (128, 128), "ident")
            iota_f = _load_w(nc, const, c_iota, (128, 128), "iota_f")
            ones_col = const.tile([128, 1], F32, tag="ones_col")
            nc.vector.memset(ones_col[:], 1.0)
            ones_row = const.tile([1, 128], F32, tag="ones_row")
            nc.vector.memset(ones_row[:], 1.0)

            # DRAM buffers for the halo exchange (AllGather outputs live in
            # the Shared scratchpad: NRT's fast HBM-HBM collective path)
            bufB = dramp.tile([nprow, WB], F32, tag="bufB")
            gathB = nc.dram_tensor("gathB_sh", [gfull, WB], F32,
                                   addr_space="Shared")
            bufC = dramp.tile([nprow, WC], F32, tag="bufC")
            gathC = nc.dram_tensor("gathC_sh", [gfull, WC], F32,
                                   addr_space="Shared")
            adst = dramp.tile([npad, H1], F32, tag="adst")
            a2dst = dramp.tile([npad, 1], F32, tag="a2dst")

            # ---- weights to SBUF
            W = {}
            for nm, shp in [
                ("vcat", (D, 2 * H1)),
                ("gw1", (D, D)), ("gb1", (1, D)), ("gw2", (D, 3)), ("gb2", (1, 3)),
                ("gcn_w1", (D, D)), ("gcn1_s", (D, 1)), ("gcn1_b", (D, 1)),
                ("sage_wl1", (D, D)), ("sage_wr1", (D, D)), ("sage_bl1", (D, 1)),
                ("w2A", (128, D)), ("w2B", (128, D)), ("v2u2", (128, 4)),
                ("w1h", (D, 4 * D)), ("b1c", (128, 2)),
                ("gcn_w2", (D, D)), ("gcn_b2c", (D, 1)),
                ("sage_wl2", (D, D)), ("sage_wr2", (D, D)),
                ("sage_bl2c", (D, 1)), ("gat_b2r", (1, D)),
            ]:
                W[nm] = _load_w(nc, wts, dr[nm], shp, nm)

            # ---- streams to SBUF (f16 halves the tunnel bytes; convert once)
            S = {}
            for nm in ("idx_row", "idx_colL"):
                t = stream.tile([128, T], I32, tag=nm)
                nc.sync.dma_start(out=t[:], in_=dr[nm][:])
                S[nm] = t
            for nm in ("colrel", "wnorm", "wsage"):
                raw = stream.tile([128, T], F16, tag=nm + "_raw")
                nc.sync.dma_start(out=raw[:], in_=dr[nm][:])
                t = stream.tile([128, T], F32, tag=nm)
                nc.vector.tensor_copy(t[:], raw[:])
                S[nm] = t

            # ---- staging buffers (SBUF-resident across phases)
            st_hs = stage.tile([128, nw * D], F32, tag="st_hs")
            st_out = stage.tile([128, nw * D], F16, tag="st_out")

            # ================================================== phase A
            with (
                tc.tile_pool(name="sbA", bufs=3) as sbA,
                tc.tile_pool(name="psA", bufs=2, space="PSUM") as psA,
                tc.tile_pool(name="pcsA", bufs=1, space="PSUM") as pcsA,
            ):
                csum_p = pcsA.tile([1, D], F32, tag="csum")
                for w in range(nw):
                    xt0 = sbA.tile([128, D + 1], F16, tag="xt0")
                    nc.sync.dma_start(out=xt0[:],
                                      in_=dr["xs65"][w * 128:(w + 1) * 128, :])
                    xt = sbA.tile([128, D + 1], F32, tag="xt")
                    nc.vector.tensor_copy(xt[:], xt0[:])
                    nc.sync.dma_start(
                        out=bufB[w * 128:(w + 1) * 128, 0:D + 1], in_=xt[:])
                    pT = psA.tile([D, 128], F32, tag="pT")
                    nc.tensor.matmul(out=pT[:], lhsT=xt[:, 0:D], rhs=ident[:],
                                     is_transpose=True)
                    xT = sbA.tile([D, 128], F32, tag="xT")
                    nc.vector.tensor_copy(xT[:], pT[:])
                    pa = psA.tile([2 * H1, 128], F32, tag="pa")
                    nc.tensor.matmul(out=pa[:], lhsT=W["vcat"][:], rhs=xT[:])
                    aT = sbA.tile([2 * H1, 128], F32, tag="aT")
                    nc.vector.tensor_copy(aT[:], pa[:])
                    pb = psA.tile([128, 2 * H1], F32, tag="pb")
                    nc.tensor.matmul(out=pb[:], lhsT=aT[:],
                                     rhs=ident[:2 * H1, :2 * H1],
                                     is_transpose=True)
                    ab = sbA.tile([128, 2 * H1], F32, tag="ab")
                    nc.vector.tensor_copy(ab[:], pb[:])
                    nc.sync.dma_start(
                        out=bufB[w * 128:(w + 1) * 128, D + 1:WB],
                        in_=ab[:, 0:H1])
                    nc.sync.dma_start(
                        out=adst[w * 128:(w + 1) * 128, :], in_=ab[:, H1:2 * H1])
                    nc.tensor.matmul(out=csum_p[:], lhsT=ones_col[:],
                                     rhs=xt[:, 0:D],
                                     start=(w == 0), stop=(w == nw - 1))
                cs = sbA.tile([1, D], F32, tag="cs")
                nc.vector.tensor_copy(cs[:], csum_p[:])
                nc.sync.dma_start(out=bufB[npad:npad + 1, 0:D], in_=cs[:])

            # ============================================== AllGather #1
            nc.gpsimd.collective_compute(
                "AllGather", ALU.bypass,
                replica_groups=[list(range(NC_N))],
                ins=[bufB[:].opt()], outs=[gathB[:].opt()])

            # ================================================== phase B
            with (
                tc.tile_pool(name="gatB", bufs=8) as gat,
                tc.tile_pool(name="mB", bufs=8) as mpool,
                tc.tile_pool(name="smB", bufs=3) as sm,
                tc.tile_pool(name="tlB", bufs=4) as tl,
                tc.tile_pool(name="paccB", bufs=1, space="PSUM") as pacc,
                tc.tile_pool(name="ptmpB", bufs=2, space="PSUM") as ptmp,
            ):
                # ---- gate MLP from the 8 AllGathered csum rows
                cs8l = sm.tile([NC_N, D], F32, tag="g_cs8l")
                for k in range(NC_N):
                    nc.sync.dma_start(
                        out=cs8l[k:k + 1, :],
                        in_=gathB[k * nprow + npad:k * nprow + npad + 1, 0:D])
                cs8 = sm.tile([NC_N, D], F32, tag="g_cs8")
                nc.vector.tensor_copy(cs8[:], cs8l[:])
                pxb = ptmp.tile([1, D], F32, tag="pt")
                nc.tensor.matmul(out=pxb[:], lhsT=ones_col[:NC_N, :1],
                                 rhs=cs8[:])
                xbar = sm.tile([1, D], F32, tag="g_xbar")
                nc.vector.tensor_scalar(out=xbar[:], in0=pxb[:],
                                        scalar1=1.0 / n_nodes, scalar2=None,
                                        op0=ALU.mult)
                pxT = ptmp.tile([D, 1], F32, tag="pt")
                nc.tensor.matmul(out=pxT[:], lhsT=xbar[:], rhs=ident[:1, :1],
                                 is_transpose=True)
                xbT = sm.tile([D, 1], F32, tag="g_xbT")
                nc.vector.tensor_copy(xbT[:], pxT[:])
                pg1 = ptmp.tile([1, D], F32, tag="pt")
                nc.tensor.matmul(out=pg1[:], lhsT=xbT[:], rhs=W["gw1"][:])
                g1 = sm.tile([1, D], F32, tag="g_g1")
                nc.vector.tensor_tensor(out=g1[:], in0=pg1[:], in1=W["gb1"][:],
                                        op=ALU.add)
                g1r = sm.tile([1, D], F32, tag="g_g1r")
                nc.vector.tensor_scalar(out=g1r[:], in0=g1[:], scalar1=0.0,
                                        scalar2=None, op0=ALU.max)
                pg1T = ptmp.tile([D, 1], F32, tag="pt")
                nc.tensor.matmul(out=pg1T[:], lhsT=g1r[:], rhs=ident[:1, :1],
                                 is_transpose=True)
                g1T = sm.tile([D, 1], F32, tag="g_g1T")
                nc.vector.tensor_copy(g1T[:], pg1T[:])
                pg2 = ptmp.tile([1, 3], F32, tag="pt")
                nc.tensor.matmul(out=pg2[:], lhsT=g1T[:], rhs=W["gw2"][:])
                g2 = sm.tile([1, 3], F32, tag="g_g2")
                nc.vector.tensor_tensor(out=g2[:], in0=pg2[:], in1=W["gb2"][:],
                                        op=ALU.add)
                g2e = sm.tile([1, 3], F32, tag="g_g2e")
                nc.scalar.activation(out=g2e[:], in_=g2[:], func=AF.Exp)
                g2s = sm.tile([1, 1], F32, tag="g_g2s")
                nc.vector.tensor_reduce(out=g2s[:], in_=g2e[:],
                                        axis=mybir.AxisListType.X, op=ALU.add)
                g2r = sm.tile([1, 1], F32, tag="g_g2r")
                nc.vector.reciprocal(g2r[:], g2s[:])
                gate_sb = sm.tile([1, 3], F32, tag="g_gate")
                nc.vector.tensor_scalar(out=gate_sb[:], in0=g2e[:],
                                        scalar1=g2r[:, :1], scalar2=None,
                                        op0=ALU.mult)
                # gate scalar broadcasts (used by phase C tails)
                pw128 = ptmp.tile([128, 3], F32, tag="pt")
                nc.tensor.matmul(out=pw128[:], lhsT=ones_row[:], rhs=gate_sb[:])
                wc = wts.tile([128, 3], F32, tag="wc")
                nc.vector.tensor_copy(wc[:], pw128[:])
                pw64 = ptmp.tile([D, 3], F32, tag="pt")
                nc.tensor.matmul(out=pw64[:], lhsT=ones_row[:1, :D],
                                 rhs=gate_sb[:])
                w64 = wts.tile([D, 3], F32, tag="w64")
                nc.vector.tensor_copy(w64[:], pw64[:])
                b2w0 = wts.tile([D, 1], F32, tag="b2w0")
                nc.vector.tensor_scalar(out=b2w0[:], in0=W["gcn_b2c"][:],
                                        scalar1=w64[:, 0:1], scalar2=None,
                                        op0=ALU.mult)
                pbg = ptmp.tile([128, D], F32, tag="pt")
                nc.tensor.matmul(out=pbg[:], lhsT=ones_row[:],
                                 rhs=W["gat_b2r"][:])
                bgat = wts.tile([128, D], F32, tag="bgat")
                nc.vector.tensor_scalar(out=bgat[:], in0=pbg[:],
                                        scalar1=wc[:, 1:2], scalar2=None,
                                        op0=ALU.mult)

                # ---- edge loop: one shared one-hot per tile, row-scaled rhs
                # p_all[dest, :] = sum_e onehot(dest)_e * [x*wn | x*ws | (x|1)*e_h ...]
                GAW = 2 * D + H1 * (D + 1)      # 388
                t_glob = 0
                for w in range(nw):
                    ntw = tiles_w[w]
                    p_all = pacc.tile([128, GAW], F32, tag="p_all")
                    for t in range(ntw):
                        Gt = gat.tile([128, WB], F32, tag="G")
                        nc.gpsimd.indirect_dma_start(
                            out=Gt[:], out_offset=None, in_=gathB[:],
                            in_offset=bass.IndirectOffsetOnAxis(
                                ap=S["idx_row"][:, t_glob:t_glob + 1], axis=0))
                        sbt = gat.tile([128, H1], F32, tag="sbt")
                        nc.gpsimd.indirect_dma_start(
                            out=sbt[:], out_offset=None, in_=adst[:],
                            in_offset=bass.IndirectOffsetOnAxis(
                                ap=S["idx_colL"][:, t_glob:t_glob + 1], axis=0))
                        zt = gat.tile([128, H1], F32, tag="z")
                        nc.vector.tensor_tensor(
                            out=zt[:], in0=Gt[:, D + 1:WB], in1=sbt[:],
                            op=ALU.add)
                        zs = gat.tile([128, H1], F32, tag="zs")
                        nc.vector.tensor_scalar(out=zs[:], in0=zt[:],
                                                scalar1=NEG_SLOPE, scalar2=None,
                                                op0=ALU.mult)
                        nc.vector.tensor_tensor(out=zt[:], in0=zt[:], in1=zs[:],
                                                op=ALU.max)
                        et = gat.tile([128, H1], F32, tag="E")
                        nc.scalar.activation(out=et[:], in_=zt[:], func=AF.Exp)

                        cr = S["colrel"][:, t_glob:t_glob + 1]
                        st, sp = (t == 0), (t == ntw - 1)
                        M0 = mpool.tile([128, 128], F32, tag="M0")
                        nc.vector.tensor_scalar(
                            out=M0[:], in0=iota_f[:], scalar1=cr,
                            scalar2=None, op0=ALU.is_equal)
                        GA = mpool.tile([128, GAW], F32, tag="GA")
                        nc.vector.tensor_scalar(
                            out=GA[:, 0:D], in0=Gt[:, 0:D],
                            scalar1=S["wnorm"][:, t_glob:t_glob + 1],
                            scalar2=None, op0=ALU.mult)
                        nc.vector.tensor_scalar(
                            out=GA[:, D:2 * D], in0=Gt[:, 0:D],
                            scalar1=S["wsage"][:, t_glob:t_glob + 1],
                            scalar2=None, op0=ALU.mult)
                        for h in range(H1):
                            nc.vector.tensor_scalar(
                                out=GA[:, 2 * D + h * (D + 1):
                                       2 * D + (h + 1) * (D + 1)],
                                in0=Gt[:, 0:D + 1],
                                scalar1=et[:, h:h + 1],
                                scalar2=None, op0=ALU.mult)
                        nc.tensor.matmul(out=p_all[:], lhsT=M0[:], rhs=GA[:],
                                         start=st, stop=sp)
                        t_glob += 1

                    # ---------- window tails ----------
                    rows = slice(w * 128, (w + 1) * 128)
                    # GCN1: h1 = relu(s*(W1^T aggT) + b)
                    aggC = tl.tile([128, D], F32, tag="aggC")
                    nc.vector.tensor_copy(aggC[:], p_all[:, 0:D])
                    paT = ptmp.tile([D, 128], F32, tag="pt")
                    nc.tensor.matmul(out=paT[:], lhsT=aggC[:], rhs=ident[:],
                                     is_transpose=True)
                    aggT = tl.tile([D, 128], F32, tag="aggT")
                    nc.vector.tensor_copy(aggT[:], paT[:])
                    ph1T = ptmp.tile([D, 128], F32, tag="pt")
                    nc.tensor.matmul(out=ph1T[:], lhsT=W["gcn_w1"][:],
                                     rhs=aggT[:])
                    h1Ts = tl.tile([D, 128], F32, tag="h1Ts")
                    nc.scalar.activation(out=h1Ts[:], in_=ph1T[:], func=AF.Relu,
                                         scale=W["gcn1_s"][:, :1],
                                         bias=W["gcn1_b"][:, :1])
                    h1Tv = tl.tile([D, 128], F32, tag="h1Tv")
                    nc.vector.tensor_copy(h1Tv[:], h1Ts[:])
                    ph1 = ptmp.tile([128, D], F32, tag="pt")
                    nc.tensor.matmul(out=ph1[:], lhsT=h1Tv[:], rhs=ident[:D, :D],
                                     is_transpose=True)
                    h1sb = tl.tile([128, D], F32, tag="h1sb")
                    nc.vector.tensor_copy(h1sb[:], ph1[:])
                    nc.sync.dma_start(out=bufC[rows, 0:D], in_=h1sb[:])

                    # GAT1 heads -> x2T halves -> h2, a2
                    x2TA = tl.tile([128, 128], F32, tag="x2TA")
                    x2TB = tl.tile([128, 128], F32, tag="x2TB")
                    for h in range(H1):
                        hb = 2 * D + h * (D + 1)
                        rd = tl.tile([128, 1], F32, tag="rd")
                        nc.vector.reciprocal(rd[:], p_all[:, hb + D:hb + D + 1])
                        hd_sb = tl.tile([128, D], F32, tag="hd_sb")
                        nc.vector.tensor_scalar(
                            out=hd_sb[:], in0=p_all[:, hb:hb + D],
                            scalar1=rd[:, :1], scalar2=None, op0=ALU.mult)
                        pht = ptmp.tile([D, 128], F32, tag="pt")
                        nc.tensor.matmul(out=pht[:], lhsT=hd_sb[:], rhs=ident[:],
                                         is_transpose=True)
                        hdT = tl.tile([D, 128], F32, tag="hdT_g")
                        nc.vector.tensor_copy(hdT[:], pht[:])
                        pxh = ptmp.tile([D, 128], F32, tag="pt")
                        nc.tensor.matmul(out=pxh[:],
                                         lhsT=W["w1h"][:, h * D:(h + 1) * D],
                                         rhs=hdT[:])
                        stgt = x2TA if h < 2 else x2TB
                        nc.vector.tensor_copy(
                            stgt[(h % 2) * D:(h % 2 + 1) * D, :], pxh[:])
                    x2T = []
                    for half, px in enumerate((x2TA, x2TB)):
                        yT = tl.tile([128, 128], F32, tag="yT")
                        nc.vector.tensor_scalar(
                            out=yT[:], in0=px[:],
                            scalar1=W["b1c"][:, half:half + 1], scalar2=None,
                            op0=ALU.add)
                        ymin = tl.tile([128, 128], F32, tag="ymin")
                        nc.vector.tensor_scalar(out=ymin[:], in0=yT[:],
                                                scalar1=0.0, scalar2=None,
                                                op0=ALU.min)
                        yexp = tl.tile([128, 128], F32, tag="yexp")
                        nc.scalar.activation(out=yexp[:], in_=ymin[:],
                                             func=AF.Exp)
                        ye1 = tl.tile([128, 128], F32, tag="ye1")
                        nc.vector.tensor_scalar(out=ye1[:], in0=yexp[:],
                                                scalar1=-1.0, scalar2=None,
                                                op0=ALU.add)
                        ymax = tl.tile([128, 128], F32, tag="ymax")
                        nc.vector.tensor_scalar(out=ymax[:], in0=yT[:],
                                                scalar1=0.0, scalar2=None,
                                                op0=ALU.max)
                        xt2 = tl.tile([128, 128], F32, tag=f"x2T{half}")
                        nc.vector.tensor_tensor(out=xt2[:], in0=ymax[:],
                                                in1=ye1[:], op=ALU.add)
                        x2T.append(xt2)
                    ph2T = ptmp.tile([D, 128], F32, tag="pt")
                    nc.tensor.matmul(out=ph2T[:], lhsT=W["w2A"][:],
                                     rhs=x2T[0][:], start=True, stop=False)
                    nc.tensor.matmul(out=ph2T[:], lhsT=W["w2B"][:],
                                     rhs=x2T[1][:], start=False, stop=True)
                    pa2T = ptmp.tile([2, 128], F32, tag="pt")
                    nc.tensor.matmul(out=pa2T[:], lhsT=W["v2u2"][:, 0:2],
                                     rhs=x2T[0][:], start=True, stop=False)
                    nc.tensor.matmul(out=pa2T[:], lhsT=W["v2u2"][:, 2:4],
                                     rhs=x2T[1][:], start=False, stop=True)
                    h2Ts = tl.tile([D, 128], F32, tag="h2Ts")
                    nc.vector.tensor_copy(h2Ts[:], ph2T[:])
                    ph2 = ptmp.tile([128, D], F32, tag="pt")
                    nc.tensor.matmul(out=ph2[:], lhsT=h2Ts[:], rhs=ident[:D, :D],
                                     is_transpose=True)
                    h2sb = tl.tile([128, D], F32, tag="h2sb")
                    nc.vector.tensor_copy(h2sb[:], ph2[:])
                    nc.sync.dma_start(out=bufC[rows, D:2 * D], in_=h2sb[:])
                    nc.sync.dma_start(out=bufC[rows, 2 * D:2 * D + 1],
                                      in_=ones_col[:])
                    a2Ts = tl.tile([2, 128], F32, tag="a2Ts")
                    nc.vector.tensor_copy(a2Ts[:], pa2T[:])
                    pa2 = ptmp.tile([128, 2], F32, tag="pt")
                    nc.tensor.matmul(out=pa2[:], lhsT=a2Ts[:], rhs=ident[:2, :2],
                                     is_transpose=True)
                    a2sb = tl.tile([128, 2], F32, tag="a2sb")
                    nc.vector.tensor_copy(a2sb[:], pa2[:])
                    nc.sync.dma_start(out=bufC[rows, WC - 1:WC],
                                      in_=a2sb[:, 0:1])
                    nc.sync.dma_start(out=a2dst[rows, :], in_=a2sb[:, 1:2])

                    # SAGE1
                    meanC = tl.tile([128, D], F32, tag="meanC")
                    nc.vector.tensor_copy(meanC[:], p_all[:, D:2 * D])
                    pmT = ptmp.tile([D, 128], F32, tag="pt")
                    nc.tensor.matmul(out=pmT[:], lhsT=meanC[:], rhs=ident[:],
                                     is_transpose=True)
                    meanT = tl.tile([D, 128], F32, tag="meanT")
                    nc.vector.tensor_copy(meanT[:], pmT[:])
                    xd0 = tl.tile([128, D], F16, tag="xd0")
                    nc.sync.dma_start(out=xd0[:], in_=dr["xs65"][rows, 0:D])
                    xd = tl.tile([128, D], F32, tag="xd")
                    nc.vector.tensor_copy(xd[:], xd0[:])
                    pxdT = ptmp.tile([D, 128], F32, tag="pt")
                    nc.tensor.matmul(out=pxdT[:], lhsT=xd[:], rhs=ident[:],
                                     is_transpose=True)
                    xdT = tl.tile([D, 128], F32, tag="xdT")
                    nc.vector.tensor_copy(xdT[:], pxdT[:])
                    psT = ptmp.tile([D, 128], F32, tag="pt")
                    nc.tensor.matmul(out=psT[:], lhsT=W["sage_wl1"][:],
                                     rhs=meanT[:], start=True, stop=False)
                    nc.tensor.matmul(out=psT[:], lhsT=W["sage_wr1"][:],
                                     rhs=xdT[:], start=False, stop=True)
                    sTs = tl.tile([D, 128], F32, tag="sTs")
                    nc.scalar.activation(out=sTs[:], in_=psT[:],
                                         func=AF.Identity,
                                         bias=W["sage_bl1"][:, :1])
                    sTv = tl.tile([D, 128], F32, tag="sTv")
                    nc.vector.tensor_copy(sTv[:], sTs[:])
                    ps_ = ptmp.tile([128, D], F32, tag="pt")
                    nc.tensor.matmul(out=ps_[:], lhsT=sTv[:], rhs=ident[:D, :D],
                                     is_transpose=True)
                    s_sb = tl.tile([128, D], F32, tag="s_sb")
                    nc.vector.tensor_copy(s_sb[:], ps_[:])
                    sq = tl.tile([128, D], F32, tag="sq")
                    nc.vector.tensor_tensor(out=sq[:], in0=s_sb[:], in1=s_sb[:],
                                            op=ALU.mult)
                    ssum = tl.tile([128, 1], F32, tag="ssum")
                    nc.vector.tensor_reduce(out=ssum[:], in_=sq[:],
                                            axis=mybir.AxisListType.X,
                                            op=ALU.add)
                    nc.vector.tensor_scalar(out=ssum[:], in0=ssum[:],
                                            scalar1=1e-24, scalar2=None,
                                            op0=ALU.add)
                    rs = tl.tile([128, 1], F32, tag="rs")
                    nc.vector.reciprocal(rs[:], ssum[:])
                    rq = tl.tile([128, 1], F32, tag="rq")
                    nc.scalar.activation(out=rq[:], in_=rs[:], func=AF.Sqrt)
                    nc.vector.tensor_scalar(out=st_hs[:, w * D:(w + 1) * D],
                                            in0=s_sb[:], scalar1=rq[:, :1],
                                            scalar2=0.0, op0=ALU.mult,
                                            op1=ALU.max)
                    nc.sync.dma_start(out=bufC[rows, 2 * D + 1:3 * D + 1],
                                      in_=st_hs[:, w * D:(w + 1) * D])

            # ============================================== AllGather #2
            nc.gpsimd.collective_compute(
                "AllGather", ALU.bypass,
                replica_groups=[list(range(NC_N))],
                ins=[bufC[:].opt()], outs=[gathC[:].opt()])

            # ================================================== phase C
            with (
                tc.tile_pool(name="gatC", bufs=8) as gat,
                tc.tile_pool(name="mC", bufs=8) as mpool,
                tc.tile_pool(name="tlC", bufs=4) as tl,
                tc.tile_pool(name="paccC", bufs=1, space="PSUM") as pacc,
                tc.tile_pool(name="ptmpC", bufs=2, space="PSUM") as ptmp,
            ):
                GW2 = 2 * D + D + 1             # 193
                t_glob = 0
                for w in range(nw):
                    ntw = tiles_w[w]
                    p_all = pacc.tile([128, GW2], F32, tag="p_all2")
                    for t in range(ntw):
                        Gt = gat.tile([128, WC], F32, tag="G2")
                        nc.gpsimd.indirect_dma_start(
                            out=Gt[:], out_offset=None, in_=gathC[:],
                            in_offset=bass.IndirectOffsetOnAxis(
                                ap=S["idx_row"][:, t_glob:t_glob + 1], axis=0))
                        sbt = gat.tile([128, 1], F32, tag="sb2")
                        nc.gpsimd.indirect_dma_start(
                            out=sbt[:], out_offset=None, in_=a2dst[:],
                            in_offset=bass.IndirectOffsetOnAxis(
                                ap=S["idx_colL"][:, t_glob:t_glob + 1], axis=0))
                        z2 = gat.tile([128, 1], F32, tag="z2")
                        nc.vector.tensor_tensor(
                            out=z2[:], in0=Gt[:, WC - 1:WC], in1=sbt[:],
                            op=ALU.add)
                        z2s = gat.tile([128, 1], F32, tag="z2s")
                        nc.vector.tensor_scalar(out=z2s[:], in0=z2[:],
                                                scalar1=NEG_SLOPE, scalar2=None,
                                                op0=ALU.mult)
                        nc.vector.tensor_tensor(out=z2[:], in0=z2[:], in1=z2s[:],
                                                op=ALU.max)
                        e2 = gat.tile([128, 1], F32, tag="E2")
                        nc.scalar.activation(out=e2[:], in_=z2[:], func=AF.Exp)

                        cr = S["colrel"][:, t_glob:t_glob + 1]
                        st, sp = (t == 0), (t == ntw - 1)
                        M0 = mpool.tile([128, 128], F32, tag="M0")
                        nc.vector.tensor_scalar(
                            out=M0[:], in0=iota_f[:], scalar1=cr,
                            scalar2=None, op0=ALU.is_equal)
                        GA = mpool.tile([128, GW2], F32, tag="GA2")
                        nc.vector.tensor_scalar(
                            out=GA[:, 0:D], in0=Gt[:, 0:D],
                            scalar1=S["wnorm"][:, t_glob:t_glob + 1],
                            scalar2=None, op0=ALU.mult)
                        nc.vector.tensor_scalar(
                            out=GA[:, D:2 * D], in0=Gt[:, 2 * D + 1:3 * D + 1],
                            scalar1=S["wsage"][:, t_glob:t_glob + 1],
                            scalar2=None, op0=ALU.mult)
                        nc.vector.tensor_scalar(
                            out=GA[:, 2 * D:GW2], in0=Gt[:, D:2 * D + 1],
                            scalar1=e2[:, 0:1],
                            scalar2=None, op0=ALU.mult)
                        nc.tensor.matmul(out=p_all[:], lhsT=M0[:], rhs=GA[:],
                                         start=st, stop=sp)
                        t_glob += 1

                    # ---------- window tails ----------
                    # GCN2 (+w0, +w0*b2)
                    aggC = tl.tile([128, D], F32, tag="aggC")
                    nc.vector.tensor_copy(aggC[:], p_all[:, 0:D])
                    paT = ptmp.tile([D, 128], F32, tag="pt")
                    nc.tensor.matmul(out=paT[:], lhsT=aggC[:], rhs=ident[:],
                                     is_transpose=True)
                    aggT = tl.tile([D, 128], F32, tag="aggT")
                    nc.vector.tensor_copy(aggT[:], paT[:])
                    poT = ptmp.tile([D, 128], F32, tag="pt")
                    nc.tensor.matmul(out=poT[:], lhsT=W["gcn_w2"][:],
                                     rhs=aggT[:])
                    oTs = tl.tile([D, 128], F32, tag="oTs")
                    nc.scalar.activation(out=oTs[:], in_=poT[:],
                                         func=AF.Identity,
                                         scale=w64[:, 0:1], bias=b2w0[:, :1])
                    oTv = tl.tile([D, 128], F32, tag="oTv")
                    nc.vector.tensor_copy(oTv[:], oTs[:])
                    po = ptmp.tile([128, D], F32, tag="pt")
                    nc.tensor.matmul(out=po[:], lhsT=oTv[:], rhs=ident[:D, :D],
                                     is_transpose=True)
                    ogcn = tl.tile([128, D], F32, tag="ogcn")
                    nc.vector.tensor_copy(ogcn[:], po[:])

                    # GAT2 (+w1)
                    rd = tl.tile([128, 1], F32, tag="rd")
                    nc.vector.reciprocal(rd[:], p_all[:, 3 * D:3 * D + 1])
                    ogat = tl.tile([128, D], F32, tag="ogat")
                    nc.vector.tensor_scalar(out=ogat[:],
                                            in0=p_all[:, 2 * D:3 * D],
                                            scalar1=rd[:, :1],
                                            scalar2=wc[:, 1:2],
                                            op0=ALU.mult, op1=ALU.mult)

                    # SAGE2 (+w2); root rows come from the SBUF staging
                    meanC = tl.tile([128, D], F32, tag="meanC")
                    nc.vector.tensor_copy(meanC[:], p_all[:, D:2 * D])
                    pmT = ptmp.tile([D, 128], F32, tag="pt")
                    nc.tensor.matmul(out=pmT[:], lhsT=meanC[:], rhs=ident[:],
                                     is_transpose=True)
                    meanT = tl.tile([D, 128], F32, tag="meanT")
                    nc.vector.tensor_copy(meanT[:], pmT[:])
                    phdT = ptmp.tile([D, 128], F32, tag="pt")
                    nc.tensor.matmul(out=phdT[:],
                                     lhsT=st_hs[:, w * D:(w + 1) * D],
                                     rhs=ident[:], is_transpose=True)
                    hdT = tl.tile([D, 128], F32, tag="hdT")
                    nc.vector.tensor_copy(hdT[:], phdT[:])
                    psT = ptmp.tile([D, 128], F32, tag="pt")
                    nc.tensor.matmul(out=psT[:], lhsT=W["sage_wl2"][:],
                                     rhs=meanT[:], start=True, stop=False)
                    nc.tensor.matmul(out=psT[:], lhsT=W["sage_wr2"][:],
                                     rhs=hdT[:], start=False, stop=True)
                    sTs = tl.tile([D, 128], F32, tag="sTs")
                    nc.scalar.activation(out=sTs[:], in_=psT[:],
                                         func=AF.Identity,
                                         bias=W["sage_bl2c"][:, :1])
                    sTv = tl.tile([D, 128], F32, tag="sTv")
                    nc.vector.tensor_copy(sTv[:], sTs[:])
                    ps_ = ptmp.tile([128, D], F32, tag="pt")
                    nc.tensor.matmul(out=ps_[:], lhsT=sTv[:], rhs=ident[:D, :D],
                                     is_transpose=True)
                    s_sb = tl.tile([128, D], F32, tag="s_sb")
                    nc.vector.tensor_copy(s_sb[:], ps_[:])
                    sq = tl.tile([128, D], F32, tag="sq")
                    nc.vector.tensor_tensor(out=sq[:], in0=s_sb[:], in1=s_sb[:],
                                            op=ALU.mult)
                    ssum = tl.tile([128, 1], F32, tag="ssum")
                    nc.vector.tensor_reduce(out=ssum[:], in_=sq[:],
                                            axis=mybir.AxisListType.X,
                                            op=ALU.add)
                    nc.vector.tensor_scalar(out=ssum[:], in0=ssum[:],
                                            scalar1=1e-24, scalar2=None,
                                            op0=ALU.add)
                    rs = tl.tile([128, 1], F32, tag="rs")
                    nc.vector.reciprocal(rs[:], ssum[:])
                    rq = tl.tile([128, 1], F32, tag="rq")
                    nc.scalar.activation(out=rq[:], in_=rs[:], func=AF.Sqrt)
                    osage = tl.tile([128, D], F32, tag="osage")
                    nc.vector.tensor_scalar(out=osage[:], in0=s_sb[:],
                                            scalar1=rq[:, :1],
                                            scalar2=wc[:, 2:3],
                                            op0=ALU.mult, op1=ALU.mult)

                    # mix
                    mx1 = tl.tile([128, D], F32, tag="mx1")
                    nc.vector.tensor_tensor(out=mx1[:], in0=ogcn[:],
                                            in1=ogat[:], op=ALU.add)
                    mx2 = tl.tile([128, D], F32, tag="mx2")
                    nc.vector.tensor_tensor(out=mx2[:], in0=mx1[:],
                                            in1=osage[:], op=ALU.add)
                    nc.vector.tensor_tensor(out=st_out[:, w * D:(w + 1) * D],
                                            in0=mx2[:], in1=bgat[:],
                                            op=ALU.add)

            # ---- final output DMA: full windows, then the partial tail
            out_ap = bass.AP(out, 0, [[D, 128], [128 * D, nw - 1], [1, D]])
            nc.sync.dma_start(
                out=out_ap,
                in_=st_out[:, 0:(nw - 1) * D].rearrange(
                    "p (w c) -> p w c", w=nw - 1))
            nc.sync.dma_start(
                out=out[(nw - 1) * 128:shard, :],
                in_=st_out[0:rem, (nw - 1) * D:nw * D])
    return nc


# ---------------------------------------------------------------- host logic
DEBUG = {}
_PROG_CACHE = {}


def _run(nc, in_maps, trace=False):
    import time as _time
    if not nc.is_finalized():
        nc.finalize()
        # the jitted wrapper re-serializes the (frozen) module on every
        # call (~0.3s for this program); memoize on the instance
        _bir = nc.to_json_bytes()
        nc.to_json_bytes = lambda _b=_bir: _b
    t0 = _time.perf_counter()
    res = run_bass_kernel_spmd(nc, in_maps, list(range(NC_N)), trace=trace)
    DEBUG.setdefault("run_walls", []).append(_time.perf_counter() - t0)
    if res.exec_time_ns:
        DEBUG.setdefault("exec_ns", []).append(res.exec_time_ns)
    return res.results


def gnn_forward(x, edge_index, gate_w1, gate_b1, gate_w2, gate_b2,
                gcn_w1, gcn_b1, bn_gamma, bn_beta, gcn_w2, gcn_b2,
                gat_w1, gat_att_src1, gat_att_dst1, gat_b1,
                gat_w2, gat_att_src2, gat_att_dst2, gat_b2,
                sage_wl1, sage_bl1, sage_wr1, sage_wl2, sage_bl2, sage_wr2,
                trace=False):
    n_nodes = x.shape[0]
    x = np.asarray(x, np.float32)
    streams, tiles_w, T, shard, nw = build_schedule(
        np.asarray(edge_index), n_nodes)
    npad = nw * 128

    # ---- host weight folding (weights only, no data)
    w1r = np.asarray(gat_w1, np.float32).reshape(D, H1, D)
    vsrc = np.einsum("chj,hj->ch", w1r, np.asarray(gat_att_src1, np.float32))
    vdst = np.einsum("chj,hj->ch", w1r, np.asarray(gat_att_dst1, np.float32))
    vcat = np.concatenate([vsrc, vdst], axis=1).astype(np.float32)  # [64,8]
    v2 = (np.asarray(gat_w2, np.float32) @
          np.asarray(gat_att_src2, np.float32)[0])  # [256]
    u2 = (np.asarray(gat_w2, np.float32) @
          np.asarray(gat_att_dst2, np.float32)[0])
    v2u2 = np.stack([v2[:128], u2[:128], v2[128:], u2[128:]],
                    axis=1).astype(np.float32)  # [128,4]
    bn_s = (np.asarray(bn_gamma, np.float32) /
            np.sqrt(np.float32(1.0 + BN_EPS)))
    gcn1_s = bn_s.reshape(D, 1).astype(np.float32)
    gcn1_b = (bn_s * np.asarray(gcn_b1, np.float32) +
              np.asarray(bn_beta, np.float32)).reshape(D, 1).astype(np.float32)

    ck = (n_nodes, T, tuple(tiles_w))
    if ck in _PROG_CACHE:
        nc = _PROG_CACHE[ck]
    else:
        nc = build_full(n_nodes, tiles_w, T)
        _PROG_CACHE[ck] = nc

    common = {
        "vcat": vcat,
        "gw1": np.asarray(gate_w1, np.float32),
        "gb1": np.asarray(gate_b1, np.float32).reshape(1, D),
        "gw2": np.asarray(gate_w2, np.float32),
        "gb2": np.asarray(gate_b2, np.float32).reshape(1, 3),
        "gcn_w1": np.asarray(gcn_w1, np.float32),
        "gcn1_s": gcn1_s, "gcn1_b": gcn1_b,
        "sage_wl1": np.asarray(sage_wl1, np.float32),
        "sage_wr1": np.asarray(sage_wr1, np.float32),
        "sage_bl1": np.asarray(sage_bl1, np.float32).reshape(D, 1),
        "w2A": np.asarray(gat_w2, np.float32)[:128],
        "w2B": np.asarray(gat_w2, np.float32)[128:],
        "v2u2": v2u2,
        "w1h": np.asarray(gat_w1, np.float32),
        "b1c": np.asarray(gat_b1, np.float32).reshape(2, 128).T.copy(),
        "gcn_w2": np.asarray(gcn_w2, np.float32),
        "gcn_b2c": np.asarray(gcn_b2, np.float32).reshape(D, 1),
        "sage_wl2": np.asarray(sage_wl2, np.float32),
        "sage_wr2": np.asarray(sage_wr2, np.float32),
        "sage_bl2c": np.asarray(sage_bl2, np.float32).reshape(D, 1),
        "gat_b2r": np.asarray(gat_b2, np.float32).reshape(1, D),
    }
    in_maps = []
    for k in range(NC_N):
        m = dict(common)
        m.update(streams[k])
        xs65 = np.zeros((npad, D + 1), np.float16)
        xs65[:shard, :D] = x[k * shard:(k + 1) * shard]
        xs65[:shard, D] = 1.0
        m["xs65"] = xs65
        in_maps.append(m)
    res = _run(nc, in_maps, trace=trace)
    out = np.concatenate([res[k]["out"] for k in range(NC_N)], 0)
    return out.astype(np.float32)


def kernel(**inputs):
    return gnn_forward(**inputs)


# revision 27
# speedup vs baseline: 1.5207x; 1.5207x over previous
"""AdaptiveGNN (GCN+GAT+SAGE mixture) on 8 Trainium2 NeuronCores — single
NEFF launch with on-device AllGather collectives.

Strategy (destination-sharded graph parallelism, one program):
 - Core k owns nodes [k*6250, (k+1)*6250); edges (plus self-loops) are
   sorted by destination on the host into a static per-window tile
   schedule shared by all 8 cores (window = 128 destination rows).
 - Phase A (local): GAT attention projections a1 = x @ [v_src|v_dst] and
   column sums of x. Writes [x | 1 | a_src] rows plus a trailing
   column-sum row into a per-core DRAM buffer.
 - AllGather #1 ([6273, 69] per core -> [50184, 69]): every core now has
   the full graph's source features + attention sources (halo exchange).
 - Gate MLP computed redundantly on every core from the 8 column-sum rows.
 - Phase B: layer 1 of all three branches for the local destination
   shard. Per edge-tile: indirect-DMA gather of source rows from the
   AllGathered buffer, local gather of a_dst rows, one-hot "selection"
   matmuls accumulate segment sums in PSUM. Window tails produce
   h1 = relu(bn(gcn1)), h2 = elu(gat1) @ W2 (+ attn scalars), hs = sage1;
   all written into the second per-core DRAM buffer [h1|h2|1|hs|a2src].
 - AllGather #2 ([6273, 194] per core -> [50184, 194]).
 - Phase C: layer 2 of all three branches + gated mix -> final rows.
 - Host sends only the x shard, edge streams and weights (~4MB/core)
   and receives the per-core output rows; one PJRT dispatch total.
"""

import hashlib
import os
import sys

sys.path.insert(0, "/opt/trn_rl_repo")

# Persistent XLA compilation cache: run_bass_via_pjrt re-jits a fresh
# closure every call, so jax's in-memory caches (weakref-keyed) can never
# hit; the disk cache is keyed on HLO bytes and skips the whole
# walrus+neuronxcc+load pipeline on repeat calls.
os.environ.setdefault("JAX_COMPILATION_CACHE_DIR",
                      os.path.expanduser("~/.cache/jax_bass_cache"))
os.environ.setdefault("JAX_PERSISTENT_CACHE_MIN_COMPILE_TIME_SECS", "0")
os.environ.setdefault("JAX_PERSISTENT_CACHE_MIN_ENTRY_SIZE_BYTES", "0")

import numpy as np

from concourse import bacc, bass, mybir, tile
from concourse import bass2jax as _b2j
from concourse.bass_utils import run_bass_kernel_spmd
import concourse.tile_sem_assignment as _tsa

# Memoize the bass_exec HLO -> NEFF-wrapped-HLO compile step. The jitted
# wrapper is rebuilt per call, so XLA recompiles the HLO each time; without
# this the BIR verify/walrus subprocess (~1.4s for this program) runs on
# every kernel() invocation. The serialized HloModuleProto differs across
# traces only in its module-id counter, so zero it for the cache key.
_HOOK_CACHE: dict = {}
_orig_neuronx_cc_hook = _b2j.neuronx_cc_hook


def _hook_key(code):
    import libneuronxla.proto.hlo_pb2 as _hlo_pb2
    p = _hlo_pb2.HloModuleProto.FromString(bytes(code))
    p.id = 0                           # per-trace module counter
    p.ClearField("stack_frame_index")  # caller source lines
    return hashlib.sha256(p.SerializeToString()).digest()


def _caching_neuronx_cc_hook(code, code_format, platform_version, file_prefix):
    if b"bass_exec" not in code:
        return _orig_neuronx_cc_hook(code, code_format, platform_version,
                                     file_prefix)
    try:
        key = _hook_key(code)
    except Exception:
        key = hashlib.sha256(bytes(code)).digest()
    r = _HOOK_CACHE.get(key)
    if r is None:
        r = _orig_neuronx_cc_hook(code, code_format, platform_version,
                                  file_prefix)
        _HOOK_CACHE[key] = r
    return r


_b2j.neuronx_cc_hook = _caching_neuronx_cc_hook

# Reuse the jitted PJRT callable across calls. run_bass_via_pjrt builds a
# fresh closure + jax.jit per invocation, so jax's jit cache misses and the
# executable is re-created and the NEFF re-loaded onto all 8 cores every
# call. This is a faithful fork of its multi-core path with the jitted
# function memoized per finalized program; run_bass_kernel_spmd still
# drives it (falls back to the stock path on any surprise).
_PJRT_CACHE: dict = {}
_orig_run_bass_via_pjrt = _b2j.run_bass_via_pjrt


def _build_pjrt_callable(nc, n_cores):
    import jax
    from jax.experimental.shard_map import shard_map
    from jax.sharding import Mesh, PartitionSpec

    _b2j.install_neuronx_cc_hook()
    assert nc.dbg_addr is None and not nc.dbg_callbacks
    partition_name = (nc.partition_id_tensor.name
                      if nc.partition_id_tensor else None)
    in_names, out_names, out_avals = [], [], []
    for alloc in nc.m.functions[0].allocations:
        if not isinstance(alloc, mybir.MemoryLocationSet):
            continue
        name = alloc.memorylocations[0].name
        if alloc.kind == "ExternalInput":
            if name != partition_name:
                in_names.append(name)
        elif alloc.kind == "ExternalOutput":
            out_names.append(name)
            out_avals.append(jax.core.ShapedArray(
                tuple(alloc.tensor_shape), mybir.dt.np(alloc.dtype)))
    n_params = len(in_names)
    n_outs = len(out_avals)
    all_in_names = list(in_names) + list(out_names)
    if partition_name is not None:
        all_in_names.append(partition_name)
    donate = tuple(range(n_params, n_params + n_outs))

    def _body(*args):
        operands = list(args)
        if partition_name is not None:
            operands.append(_b2j.partition_id_tensor())
        return tuple(_b2j._bass_exec_p.bind(
            *operands,
            out_avals=tuple(out_avals),
            in_names=tuple(all_in_names),
            out_names=tuple(out_names),
            lowering_input_output_aliases=(),
            sim_require_finite=True,
            sim_require_nnan=True,
            nc=nc,
        ))

    devices = jax.devices()[:n_cores]
    assert len(devices) == n_cores
    mesh = Mesh(np.asarray(devices), ("core",))
    in_specs = (PartitionSpec("core"),) * (n_params + n_outs)
    out_specs = (PartitionSpec("core"),) * n_outs
    sharded = jax.jit(
        shard_map(_body, mesh=mesh, in_specs=in_specs, out_specs=out_specs,
                  check_rep=False),
        donate_argnums=donate, keep_unused=True)

    def call(in_maps):
        per_core = [[np.asarray(m[name]) for name in in_names]
                    for m in in_maps]
        concat_in = [
            np.concatenate([per_core[c][i] for c in range(n_cores)], axis=0)
            for i in range(n_params)]
        concat_zeros = [
            np.zeros((n_cores * a.shape[0], *a.shape[1:]), a.dtype)
            for a in out_avals]
        out_arrs = sharded(*concat_in, *concat_zeros)
        return [
            {name: np.asarray(out_arrs[i]).reshape(
                n_cores, *out_avals[i].shape)[c]
             for i, name in enumerate(out_names)}
            for c in range(n_cores)]

    return call


def _cached_run_bass_via_pjrt(nc, in_maps, n_cores):
    try:
        ent = _PJRT_CACHE.get(id(nc))
        if ent is None:
            # hold the nc ref in the entry so its id() stays unique
            ent = (_build_pjrt_callable(nc, n_cores), nc)
            _PJRT_CACHE[id(nc)] = ent
        return ent[0](in_maps)
    except Exception:
        return _orig_run_bass_via_pjrt(nc, in_maps, n_cores)


_b2j.run_bass_via_pjrt = _cached_run_bass_via_pjrt

# Clamp Tile's DMA-completion semaphore lanes (kernel-tail Drain waits on
# every producer semaphore; walrus rejects instructions with too many
# sync waits).
_tsa.NUM_HWDGE_SEMS = 8
_tsa.NUM_SWDGE_GLOBAL_SEMS = 8

F32 = mybir.dt.float32
F16 = mybir.dt.float16
I32 = mybir.dt.int32
AF = mybir.ActivationFunctionType
ALU = mybir.AluOpType

NC_N = 8          # cores
D = 64            # feature dim
H1 = 4            # GAT hidden heads
WB = D + 1 + H1   # phase-B gather row: [x | 1 | a_src]            = 69
WC = 3 * D + 2    # phase-C gather row: [h1 | h2 | 1 | hs | a2src] = 194
NEG_SLOPE = 0.2
BN_EPS = 1e-5


# ----------------------------------------------------------------- host prep
def build_schedule(edge_index, n_nodes):
    """Sort edges (plus self-loops) by destination, shard by destination,
    and produce a tile schedule common to all cores plus per-core streams.
    Source indices are remapped into the AllGather row space
    (node n -> (n // shard) * (npad + 1) + n % shard)."""
    shard = n_nodes // NC_N
    nw = (shard + 127) // 128
    # per-core AllGather rows: +1 csum row, +1 pad so the collective's
    # element count stays even (NRT needs 8-byte-aligned collective sizes)
    nprow = nw * 128 + 2
    row = edge_index[0].astype(np.int64)
    col = edge_index[1].astype(np.int64)
    loops = np.arange(n_nodes, dtype=np.int64)
    r_all = np.concatenate([row, loops])
    c_all = np.concatenate([col, loops])

    # GCN symmetric normalization (self-loops included)
    deg = np.bincount(c_all, minlength=n_nodes).astype(np.float64)
    dis = np.where(deg > 0, deg ** -0.5, 0.0)
    wnorm_all = (dis[r_all] * dis[c_all]).astype(np.float32)
    # SAGE mean weights (real edges only; zero on appended self-loops)
    cnt = np.bincount(col, minlength=n_nodes).astype(np.float64)
    ws = (1.0 / np.maximum(cnt, 1.0))[col].astype(np.float32)
    wsage_all = np.concatenate([ws, np.zeros(n_nodes, np.float32)])
    # source node id -> AllGathered row
    rg_all = ((r_all // shard) * nprow + (r_all % shard)).astype(np.int64)

    per_core = []
    counts = np.zeros((NC_N, nw), dtype=np.int64)
    for k in range(NC_N):
        lo, hi = k * shard, (k + 1) * shard
        sel = np.nonzero((c_all >= lo) & (c_all < hi))[0]
        cl = c_all[sel] - lo
        order = np.argsort(cl, kind="stable")
        sel = sel[order]
        cl = cl[order]
        w_of = cl // 128
        cnts = np.bincount(w_of, minlength=nw)
        counts[k] = cnts
        per_core.append((sel, cl, cnts))

    tiles_w = np.maximum(1, (counts.max(axis=0) + 127) // 128)
    T = int(tiles_w.sum())

    streams = []
    for k in range(NC_N):
        sel, cl, cnts = per_core[k]
        idx_row = np.zeros(T * 128, np.int32)
        idx_colL = np.zeros(T * 128, np.int32)
        colrel = np.full(T * 128, -1.0, np.float32)
        wnorm = np.zeros(T * 128, np.float32)
        wsage = np.zeros(T * 128, np.float32)
        pos = 0      # position in padded stream
        epos = 0     # position in this core's sorted edge list
        for w in range(nw):
            cw = int(cnts[w])
            seg = sel[epos:epos + cw]
            base = pos
            idx_row[base:base + cw] = rg_all[seg]
            idx_colL[base:base + cw] = cl[epos:epos + cw]
            colrel[base:base + cw] = (cl[epos:epos + cw] % 128).astype(np.float32)
            wnorm[base:base + cw] = wnorm_all[seg]
            wsage[base:base + cw] = wsage_all[seg]
            epos += cw
            pos += int(tiles_w[w]) * 128
        st = {
            "idx_row": idx_row.reshape(T, 128).T.copy(),
            "idx_colL": idx_colL.reshape(T, 128).T.copy(),
            "colrel": colrel.reshape(T, 128).T.astype(np.float16),
            "wnorm": wnorm.reshape(T, 128).T.astype(np.float16),
            "wsage": wsage.reshape(T, 128).T.astype(np.float16),
        }
        streams.append(st)
    return streams, [int(t) for t in tiles_w], T, shard, nw


# ------------------------------------------------------------- device pieces
def _load_w(nc, pool, dram, shape, tag, in_dtype=None):
    ld = pool.tile(list(shape), in_dtype or F32, tag=tag + "_ld")
    nc.sync.dma_start(out=ld[:], in_=dram[:])
    t = pool.tile(list(shape), F32, tag=tag)
    nc.vector.tensor_copy(t[:], ld[:])
    return t


# ----------------------------------------------------------------- the build
def build_full(n_nodes, tiles_w, T):
    shard = n_nodes // NC_N
    nw = (shard + 127) // 128
    npad = nw * 128
    nprow = npad + 2   # +1 csum row, +1 pad row (8-byte collective align)
    gfull = NC_N * nprow
    rem = shard - (nw - 1) * 128       # rows in last output window

    nc = bacc.Bacc()
    dr = {}
    for nm, shp, dt in [
        ("xs65", [npad, D + 1], F16),
        ("idx_row", [128, T], I32), ("idx_colL", [128, T], I32),
        ("colrel", [128, T], F16), ("wnorm", [128, T], F16),
        ("wsage", [128, T], F16),
        ("vcat", [D, 2 * H1], F32),
        ("gw1", [D, D], F32), ("gb1", [1, D], F32),
        ("gw2", [D, 3], F32), ("gb2", [1, 3], F32),
        ("gcn_w1", [D, D], F32), ("gcn1_s", [D, 1], F32), ("gcn1_b", [D, 1], F32),
        ("sage_wl1", [D, D], F32), ("sage_wr1", [D, D], F32),
        ("sage_bl1", [D, 1], F32),
        ("w2A", [128, D], F32), ("w2B", [128, D], F32),
        ("v2u2", [128, 4], F32), ("w1h", [D, 4 * D], F32),
        ("b1c", [128, 2], F32),
        ("gcn_w2", [D, D], F32), ("gcn_b2c", [D, 1], F32),
        ("sage_wl2", [D, D], F32), ("sage_wr2", [D, D], F32),
        ("sage_bl2c", [D, 1], F32), ("gat_b2r", [1, D], F32),
    ]:
        dr[nm] = nc.dram_tensor(nm, shp, dt, kind="ExternalInput")
    out = nc.dram_tensor("out", [shard, D], F16, kind="ExternalOutput")
    c_ident = nc.inline_tensor(np.eye(128, dtype=np.float32), name="cident")
    c_iota = nc.inline_tensor(
        np.tile(np.arange(128, dtype=np.float32), (128, 1)), name="ciota")

    with tile.TileContext(nc) as tc:
        with (
            tc.tile_pool(name="const", bufs=1) as const,
            tc.tile_pool(name="wts", bufs=1) as wts,
            tc.tile_pool(name="stream", bufs=1) as stream,
            tc.tile_pool(name="stage", bufs=1) as stage,
            tc.tile_pool(name="dramp", bufs=1, space="DRAM") as dramp,
        ):
            ident = _load_w(nc, const, c_ident, (128, 128), "ident")
            iota_f = _load_w(nc, const, c_iota, (128, 128), "iota_f")
            ones_col = const.tile([128, 1], F32, tag="ones_col")
            nc.vector.memset(ones_col[:], 1.0)
            ones_row = const.tile([1, 128], F32, tag="ones_row")
            nc.vector.memset(ones_row[:], 1.0)

            # DRAM buffers for the halo exchange (AllGather outputs live in
            # the Shared scratchpad: NRT's fast HBM-HBM collective path)
            bufB = dramp.tile([nprow, WB], F32, tag="bufB")
            gathB = nc.dram_tensor("gathB_sh", [gfull, WB], F32,
                                   addr_space="Shared")
            bufC = dramp.tile([nprow, WC], F32, tag="bufC")
            gathC = nc.dram_tensor("gathC_sh", [gfull, WC], F32,
                                   addr_space="Shared")
            adst = dramp.tile([npad, H1], F32, tag="adst")
            a2dst = dramp.tile([npad, 1], F32, tag="a2dst")

            # ---- weights to SBUF
            W = {}
            for nm, shp in [
                ("vcat", (D, 2 * H1)),
                ("gw1", (D, D)), ("gb1", (1, D)), ("gw2", (D, 3)), ("gb2", (1, 3)),
                ("gcn_w1", (D, D)), ("gcn1_s", (D, 1)), ("gcn1_b", (D, 1)),
                ("sage_wl1", (D, D)), ("sage_wr1", (D, D)), ("sage_bl1", (D, 1)),
                ("w2A", (128, D)), ("w2B", (128, D)), ("v2u2", (128, 4)),
                ("w1h", (D, 4 * D)), ("b1c", (128, 2)),
                ("gcn_w2", (D, D)), ("gcn_b2c", (D, 1)),
                ("sage_wl2", (D, D)), ("sage_wr2", (D, D)),
                ("sage_bl2c", (D, 1)), ("gat_b2r", (1, D)),
            ]:
                W[nm] = _load_w(nc, wts, dr[nm], shp, nm)

            # ---- streams to SBUF (f16 halves the tunnel bytes; convert once)
            S = {}
            for nm in ("idx_row", "idx_colL"):
                t = stream.tile([128, T], I32, tag=nm)
                nc.sync.dma_start(out=t[:], in_=dr[nm][:])
                S[nm] = t
            for nm in ("colrel", "wnorm", "wsage"):
                raw = stream.tile([128, T], F16, tag=nm + "_raw")
                nc.sync.dma_start(out=raw[:], in_=dr[nm][:])
                t = stream.tile([128, T], F32, tag=nm)
                nc.vector.tensor_copy(t[:], raw[:])
                S[nm] = t

            # ---- staging buffers (SBUF-resident across phases)
            st_hs = stage.tile([128, nw * D], F32, tag="st_hs")
            st_out = stage.tile([128, nw * D], F16, tag="st_out")

            # ================================================== phase A
            with (
                tc.tile_pool(name="sbA", bufs=3) as sbA,
                tc.tile_pool(name="psA", bufs=2, space="PSUM") as psA,
                tc.tile_pool(name="pcsA", bufs=1, space="PSUM") as pcsA,
            ):
                csum_p = pcsA.tile([1, D], F32, tag="csum")
                for w in range(nw):
                    xt0 = sbA.tile([128, D + 1], F16, tag="xt0")
                    nc.sync.dma_start(out=xt0[:],
                                      in_=dr["xs65"][w * 128:(w + 1) * 128, :])
                    xt = sbA.tile([128, D + 1], F32, tag="xt")
                    nc.vector.tensor_copy(xt[:], xt0[:])
                    nc.sync.dma_start(
                        out=bufB[w * 128:(w + 1) * 128, 0:D + 1], in_=xt[:])
                    pT = psA.tile([D, 128], F32, tag="pT")
                    nc.tensor.matmul(out=pT[:], lhsT=xt[:, 0:D], rhs=ident[:],
                                     is_transpose=True)
                    xT = sbA.tile([D, 128], F32, tag="xT")
                    nc.vector.tensor_copy(xT[:], pT[:])
                    pa = psA.tile([2 * H1, 128], F32, tag="pa")
                    nc.tensor.matmul(out=pa[:], lhsT=W["vcat"][:], rhs=xT[:])
                    aT = sbA.tile([2 * H1, 128], F32, tag="aT")
                    nc.vector.tensor_copy(aT[:], pa[:])
                    pb = psA.tile([128, 2 * H1], F32, tag="pb")
                    nc.tensor.matmul(out=pb[:], lhsT=aT[:],
                                     rhs=ident[:2 * H1, :2 * H1],
                                     is_transpose=True)
                    ab = sbA.tile([128, 2 * H1], F32, tag="ab")
                    nc.vector.tensor_copy(ab[:], pb[:])
                    nc.sync.dma_start(
                        out=bufB[w * 128:(w + 1) * 128, D + 1:WB],
                        in_=ab[:, 0:H1])
                    nc.sync.dma_start(
                        out=adst[w * 128:(w + 1) * 128, :], in_=ab[:, H1:2 * H1])
                    nc.tensor.matmul(out=csum_p[:], lhsT=ones_col[:],
                                     rhs=xt[:, 0:D],
                                     start=(w == 0), stop=(w == nw - 1))
                cs = sbA.tile([1, D], F32, tag="cs")
                nc.vector.tensor_copy(cs[:], csum_p[:])
                nc.sync.dma_start(out=bufB[npad:npad + 1, 0:D], in_=cs[:])

            # ============================================== AllGather #1
            nc.gpsimd.collective_compute(
                "AllGather", ALU.bypass,
                replica_groups=[list(range(NC_N))],
                ins=[bufB[:].opt()], outs=[gathB[:].opt()])

            # ================================================== phase B
            with (
                tc.tile_pool(name="gatB", bufs=8) as gat,
                tc.tile_pool(name="mB", bufs=8) as mpool,
                tc.tile_pool(name="smB", bufs=3) as sm,
                tc.tile_pool(name="tlB", bufs=4) as tl,
                tc.tile_pool(name="paccB", bufs=1, space="PSUM") as pacc,
                tc.tile_pool(name="ptmpB", bufs=2, space="PSUM") as ptmp,
            ):
                # ---- gate MLP from the 8 AllGathered csum rows
                cs8l = sm.tile([NC_N, D], F32, tag="g_cs8l")
                for k in range(NC_N):
                    nc.sync.dma_start(
                        out=cs8l[k:k + 1, :],
                        in_=gathB[k * nprow + npad:k * nprow + npad + 1, 0:D])
                cs8 = sm.tile([NC_N, D], F32, tag="g_cs8")
                nc.vector.tensor_copy(cs8[:], cs8l[:])
                pxb = ptmp.tile([1, D], F32, tag="pt")
                nc.tensor.matmul(out=pxb[:], lhsT=ones_col[:NC_N, :1],
                                 rhs=cs8[:])
                xbar = sm.tile([1, D], F32, tag="g_xbar")
                nc.vector.tensor_scalar(out=xbar[:], in0=pxb[:],
                                        scalar1=1.0 / n_nodes, scalar2=None,
                                        op0=ALU.mult)
                pxT = ptmp.tile([D, 1], F32, tag="pt")
                nc.tensor.matmul(out=pxT[:], lhsT=xbar[:], rhs=ident[:1, :1],
                                 is_transpose=True)
                xbT = sm.tile([D, 1], F32, tag="g_xbT")
                nc.vector.tensor_copy(xbT[:], pxT[:])
                pg1 = ptmp.tile([1, D], F32, tag="pt")
                nc.tensor.matmul(out=pg1[:], lhsT=xbT[:], rhs=W["gw1"][:])
                g1 = sm.tile([1, D], F32, tag="g_g1")
                nc.vector.tensor_tensor(out=g1[:], in0=pg1[:], in1=W["gb1"][:],
                                        op=ALU.add)
                g1r = sm.tile([1, D], F32, tag="g_g1r")
                nc.vector.tensor_scalar(out=g1r[:], in0=g1[:], scalar1=0.0,
                                        scalar2=None, op0=ALU.max)
                pg1T = ptmp.tile([D, 1], F32, tag="pt")
                nc.tensor.matmul(out=pg1T[:], lhsT=g1r[:], rhs=ident[:1, :1],
                                 is_transpose=True)
                g1T = sm.tile([D, 1], F32, tag="g_g1T")
                nc.vector.tensor_copy(g1T[:], pg1T[:])
                pg2 = ptmp.tile([1, 3], F32, tag="pt")
                nc.tensor.matmul(out=pg2[:], lhsT=g1T[:], rhs=W["gw2"][:])
                g2 = sm.tile([1, 3], F32, tag="g_g2")
                nc.vector.tensor_tensor(out=g2[:], in0=pg2[:], in1=W["gb2"][:],
                                        op=ALU.add)
                g2e = sm.tile([1, 3], F32, tag="g_g2e")
                nc.scalar.activation(out=g2e[:], in_=g2[:], func=AF.Exp)
                g2s = sm.tile([1, 1], F32, tag="g_g2s")
                nc.vector.tensor_reduce(out=g2s[:], in_=g2e[:],
                                        axis=mybir.AxisListType.X, op=ALU.add)
                g2r = sm.tile([1, 1], F32, tag="g_g2r")
                nc.vector.reciprocal(g2r[:], g2s[:])
                gate_sb = sm.tile([1, 3], F32, tag="g_gate")
                nc.vector.tensor_scalar(out=gate_sb[:], in0=g2e[:],
                                        scalar1=g2r[:, :1], scalar2=None,
                                        op0=ALU.mult)
                # gate scalar broadcasts (used by phase C tails)
                pw128 = ptmp.tile([128, 3], F32, tag="pt")
                nc.tensor.matmul(out=pw128[:], lhsT=ones_row[:], rhs=gate_sb[:])
                wc = wts.tile([128, 3], F32, tag="wc")
                nc.vector.tensor_copy(wc[:], pw128[:])
                pw64 = ptmp.tile([D, 3], F32, tag="pt")
                nc.tensor.matmul(out=pw64[:], lhsT=ones_row[:1, :D],
                                 rhs=gate_sb[:])
                w64 = wts.tile([D, 3], F32, tag="w64")
                nc.vector.tensor_copy(w64[:], pw64[:])
                b2w0 = wts.tile([D, 1], F32, tag="b2w0")
                nc.vector.tensor_scalar(out=b2w0[:], in0=W["gcn_b2c"][:],
                                        scalar1=w64[:, 0:1], scalar2=None,
                                        op0=ALU.mult)
                pbg = ptmp.tile([128, D], F32, tag="pt")
                nc.tensor.matmul(out=pbg[:], lhsT=ones_row[:],
                                 rhs=W["gat_b2r"][:])
                bgat = wts.tile([128, D], F32, tag="bgat")
                nc.vector.tensor_scalar(out=bgat[:], in0=pbg[:],
                                        scalar1=wc[:, 1:2], scalar2=None,
                                        op0=ALU.mult)

                # ---- edge loop: one shared one-hot per tile, row-scaled rhs
                # p_all[dest, :] = sum_e onehot(dest)_e * [x*wn | x*ws | (x|1)*e_h ...]
                GAW = 2 * D + H1 * (D + 1)      # 388
                t_glob = 0
                for w in range(nw):
                    ntw = tiles_w[w]
                    p_all = pacc.tile([128, GAW], F32, tag="p_all")
                    for t in range(ntw):
                        Gt = gat.tile([128, WB], F32, tag="G")
                        nc.gpsimd.indirect_dma_start(
                            out=Gt[:], out_offset=None, in_=gathB[:],
                            in_offset=bass.IndirectOffsetOnAxis(
                                ap=S["idx_row"][:, t_glob:t_glob + 1], axis=0))
                        sbt = gat.tile([128, H1], F32, tag="sbt")
                        nc.gpsimd.indirect_dma_start(
                            out=sbt[:], out_offset=None, in_=adst[:],
                            in_offset=bass.IndirectOffsetOnAxis(
                                ap=S["idx_colL"][:, t_glob:t_glob + 1], axis=0))
                        zt = gat.tile([128, H1], F32, tag="z")
                        nc.vector.tensor_tensor(
                            out=zt[:], in0=Gt[:, D + 1:WB], in1=sbt[:],
                            op=ALU.add)
                        zs = gat.tile([128, H1], F32, tag="zs")
                        nc.vector.tensor_scalar(out=zs[:], in0=zt[:],
                                                scalar1=NEG_SLOPE, scalar2=None,
                                                op0=ALU.mult)
                        nc.vector.tensor_tensor(out=zt[:], in0=zt[:], in1=zs[:],
                                                op=ALU.max)
                        et = gat.tile([128, H1], F32, tag="E")
                        nc.scalar.activation(out=et[:], in_=zt[:], func=AF.Exp)

                        cr = S["colrel"][:, t_glob:t_glob + 1]
                        st, sp = (t == 0), (t == ntw - 1)
                        M0 = mpool.tile([128, 128], F32, tag="M0")
                        nc.vector.tensor_scalar(
                            out=M0[:], in0=iota_f[:], scalar1=cr,
                            scalar2=None, op0=ALU.is_equal)
                        GA = mpool.tile([128, GAW], F32, tag="GA")
                        nc.vector.tensor_scalar(
                            out=GA[:, 0:D], in0=Gt[:, 0:D],
                            scalar1=S["wnorm"][:, t_glob:t_glob + 1],
                            scalar2=None, op0=ALU.mult)
                        nc.vector.tensor_scalar(
                            out=GA[:, D:2 * D], in0=Gt[:, 0:D],
                            scalar1=S["wsage"][:, t_glob:t_glob + 1],
                            scalar2=None, op0=ALU.mult)
                        for h in range(H1):
                            nc.vector.tensor_scalar(
                                out=GA[:, 2 * D + h * (D + 1):
                                       2 * D + (h + 1) * (D + 1)],
                                in0=Gt[:, 0:D + 1],
                                scalar1=et[:, h:h + 1],
                                scalar2=None, op0=ALU.mult)
                        nc.tensor.matmul(out=p_all[:], lhsT=M0[:], rhs=GA[:],
                                         start=st, stop=sp)
                        t_glob += 1

                    # ---------- window tails ----------
                    rows = slice(w * 128, (w + 1) * 128)
                    # GCN1: h1 = relu(s*(W1^T aggT) + b)
                    aggC = tl.tile([128, D], F32, tag="aggC")
                    nc.vector.tensor_copy(aggC[:], p_all[:, 0:D])
                    paT = ptmp.tile([D, 128], F32, tag="pt")
                    nc.tensor.matmul(out=paT[:], lhsT=aggC[:], rhs=ident[:],
                                     is_transpose=True)
                    aggT = tl.tile([D, 128], F32, tag="aggT")
                    nc.vector.tensor_copy(aggT[:], paT[:])
                    ph1T = ptmp.tile([D, 128], F32, tag="pt")
                    nc.tensor.matmul(out=ph1T[:], lhsT=W["gcn_w1"][:],
                                     rhs=aggT[:])
                    h1Ts = tl.tile([D, 128], F32, tag="h1Ts")
                    nc.scalar.activation(out=h1Ts[:], in_=ph1T[:], func=AF.Relu,
                                         scale=W["gcn1_s"][:, :1],
                                         bias=W["gcn1_b"][:, :1])
                    h1Tv = tl.tile([D, 128], F32, tag="h1Tv")
                    nc.vector.tensor_copy(h1Tv[:], h1Ts[:])
                    ph1 = ptmp.tile([128, D], F32, tag="pt")
                    nc.tensor.matmul(out=ph1[:], lhsT=h1Tv[:], rhs=ident[:D, :D],
                                     is_transpose=True)
                    h1sb = tl.tile([128, D], F32, tag="h1sb")
                    nc.vector.tensor_copy(h1sb[:], ph1[:])
                    nc.sync.dma_start(out=bufC[rows, 0:D], in_=h1sb[:])

                    # GAT1 heads -> x2T halves -> h2, a2
                    x2TA = tl.tile([128, 128], F32, tag="x2TA")
                    x2TB = tl.tile([128, 128], F32, tag="x2TB")
                    for h in range(H1):
                        hb = 2 * D + h * (D + 1)
                        rd = tl.tile([128, 1], F32, tag="rd")
                        nc.vector.reciprocal(rd[:], p_all[:, hb + D:hb + D + 1])
                        hd_sb = tl.tile([128, D], F32, tag="hd_sb")
                        nc.vector.tensor_scalar(
                            out=hd_sb[:], in0=p_all[:, hb:hb + D],
                            scalar1=rd[:, :1], scalar2=None, op0=ALU.mult)
                        pht = ptmp.tile([D, 128], F32, tag="pt")
                        nc.tensor.matmul(out=pht[:], lhsT=hd_sb[:], rhs=ident[:],
                                         is_transpose=True)
                        hdT = tl.tile([D, 128], F32, tag="hdT_g")
                        nc.vector.tensor_copy(hdT[:], pht[:])
                        pxh = ptmp.tile([D, 128], F32, tag="pt")
                        nc.tensor.matmul(out=pxh[:],
                                         lhsT=W["w1h"][:, h * D:(h + 1) * D],
                                         rhs=hdT[:])
                        stgt = x2TA if h < 2 else x2TB
                        nc.vector.tensor_copy(
                            stgt[(h % 2) * D:(h % 2 + 1) * D, :], pxh[:])
                    x2T = []
                    for half, px in enumerate((x2TA, x2TB)):
                        yT = tl.tile([128, 128], F32, tag="yT")
                        nc.vector.tensor_scalar(
                            out=yT[:], in0=px[:],
                            scalar1=W["b1c"][:, half:half + 1], scalar2=None,
                            op0=ALU.add)
                        ymin = tl.tile([128, 128], F32, tag="ymin")
                        nc.vector.tensor_scalar(out=ymin[:], in0=yT[:],
                                                scalar1=0.0, scalar2=None,
                                                op0=ALU.min)
                        yexp = tl.tile([128, 128], F32, tag="yexp")
                        nc.scalar.activation(out=yexp[:], in_=ymin[:],
                                             func=AF.Exp)
                        ye1 = tl.tile([128, 128], F32, tag="ye1")
                        nc.vector.tensor_scalar(out=ye1[:], in0=yexp[:],
                                                scalar1=-1.0, scalar2=None,
                                                op0=ALU.add)
                        ymax = tl.tile([128, 128], F32, tag="ymax")
                        nc.vector.tensor_scalar(out=ymax[:], in0=yT[:],
                                                scalar1=0.0, scalar2=None,
                                                op0=ALU.max)
                        xt2 = tl.tile([128, 128], F32, tag=f"x2T{half}")
                        nc.vector.tensor_tensor(out=xt2[:], in0=ymax[:],
                                                in1=ye1[:], op=ALU.add)
                        x2T.append(xt2)
                    ph2T = ptmp.tile([D, 128], F32, tag="pt")
                    nc.tensor.matmul(out=ph2T[:], lhsT=W["w2A"][:],
                                     rhs=x2T[0][:], start=True, stop=False)
                    nc.tensor.matmul(out=ph2T[:], lhsT=W["w2B"][:],
                                     rhs=x2T[1][:], start=False, stop=True)
                    pa2T = ptmp.tile([2, 128], F32, tag="pt")
                    nc.tensor.matmul(out=pa2T[:], lhsT=W["v2u2"][:, 0:2],
                                     rhs=x2T[0][:], start=True, stop=False)
                    nc.tensor.matmul(out=pa2T[:], lhsT=W["v2u2"][:, 2:4],
                                     rhs=x2T[1][:], start=False, stop=True)
                    h2Ts = tl.tile([D, 128], F32, tag="h2Ts")
                    nc.vector.tensor_copy(h2Ts[:], ph2T[:])
                    ph2 = ptmp.tile([128, D], F32, tag="pt")
                    nc.tensor.matmul(out=ph2[:], lhsT=h2Ts[:], rhs=ident[:D, :D],
                                     is_transpose=True)
                    h2sb = tl.tile([128, D], F32, tag="h2sb")
                    nc.vector.tensor_copy(h2sb[:], ph2[:])
                    nc.sync.dma_start(out=bufC[rows, D:2 * D], in_=h2sb[:])
                    nc.sync.dma_start(out=bufC[rows, 2 * D:2 * D + 1],
                                      in_=ones_col[:])
                    a2Ts = tl.tile([2, 128], F32, tag="a2Ts")
                    nc.vector.tensor_copy(a2Ts[:], pa2T[:])
                    pa2 = ptmp.tile([128, 2], F32, tag="pt")
                    nc.tensor.matmul(out=pa2[:], lhsT=a2Ts[:], rhs=ident[:2, :2],
                                     is_transpose=True)
                    a2sb = tl.tile([128, 2], F32, tag="a2sb")
                    nc.vector.tensor_copy(a2sb[:], pa2[:])
                    nc.sync.dma_start(out=bufC[rows, WC - 1:WC],
                                      in_=a2sb[:, 0:1])
                    nc.sync.dma_start(out=a2dst[rows, :], in_=a2sb[:, 1:2])

                    # SAGE1
                    meanC = tl.tile([128, D], F32, tag="meanC")
                    nc.vector.tensor_copy(meanC[:], p_all[:, D:2 * D])
                    pmT = ptmp.tile([D, 128], F32, tag="pt")
                    nc.tensor.matmul(out=pmT[:], lhsT=meanC[:], rhs=ident[:],
                                     is_transpose=True)
                    meanT = tl.tile([D, 128], F32, tag="meanT")
                    nc.vector.tensor_copy(meanT[:], pmT[:])
                    xd0 = tl.tile([128, D], F16, tag="xd0")
                    nc.sync.dma_start(out=xd0[:], in_=dr["xs65"][rows, 0:D])
                    xd = tl.tile([128, D], F32, tag="xd")
                    nc.vector.tensor_copy(xd[:], xd0[:])
                    pxdT = ptmp.tile([D, 128], F32, tag="pt")
                    nc.tensor.matmul(out=pxdT[:], lhsT=xd[:], rhs=ident[:],
                                     is_transpose=True)
                    xdT = tl.tile([D, 128], F32, tag="xdT")
                    nc.vector.tensor_copy(xdT[:], pxdT[:])
                    psT = ptmp.tile([D, 128], F32, tag="pt")
                    nc.tensor.matmul(out=psT[:], lhsT=W["sage_wl1"][:],
                                     rhs=meanT[:], start=True, stop=False)
                    nc.tensor.matmul(out=psT[:], lhsT=W["sage_wr1"][:],
                                     rhs=xdT[:], start=False, stop=True)
                    sTs = tl.tile([D, 128], F32, tag="sTs")
                    nc.scalar.activation(out=sTs[:], in_=psT[:],
                                         func=AF.Identity,
                                         bias=W["sage_bl1"][:, :1])
                    sTv = tl.tile([D, 128], F32, tag="sTv")
                    nc.vector.tensor_copy(sTv[:], sTs[:])
                    ps_ = ptmp.tile([128, D], F32, tag="pt")
                    nc.tensor.matmul(out=ps_[:], lhsT=sTv[:], rhs=ident[:D, :D],
                                     is_transpose=True)
                    s_sb = tl.tile([128, D], F32, tag="s_sb")
                    nc.vector.tensor_copy(s_sb[:], ps_[:])
                    sq = tl.tile([128, D], F32, tag="sq")
                    nc.vector.tensor_tensor(out=sq[:], in0=s_sb[:], in1=s_sb[:],
                                            op=ALU.mult)
                    ssum = tl.tile([128, 1], F32, tag="ssum")
                    nc.vector.tensor_reduce(out=ssum[:], in_=sq[:],
                                            axis=mybir.AxisListType.X,
                                            op=ALU.add)
                    nc.vector.tensor_scalar(out=ssum[:], in0=ssum[:],
                                            scalar1=1e-24, scalar2=None,
                                            op0=ALU.add)
                    rs = tl.tile([128, 1], F32, tag="rs")
                    nc.vector.reciprocal(rs[:], ssum[:])
                    rq = tl.tile([128, 1], F32, tag="rq")
                    nc.scalar.activation(out=rq[:], in_=rs[:], func=AF.Sqrt)
                    nc.vector.tensor_scalar(out=st_hs[:, w * D:(w + 1) * D],
                                            in0=s_sb[:], scalar1=rq[:, :1],
                                            scalar2=0.0, op0=ALU.mult,
                                            op1=ALU.max)
                    nc.sync.dma_start(out=bufC[rows, 2 * D + 1:3 * D + 1],
                                      in_=st_hs[:, w * D:(w + 1) * D])

            # ============================================== AllGather #2
            nc.gpsimd.collective_compute(
                "AllGather", ALU.bypass,
                replica_groups=[list(range(NC_N))],
                ins=[bufC[:].opt()], outs=[gathC[:].opt()])

            # ================================================== phase C
            with (
                tc.tile_pool(name="gatC", bufs=8) as gat,
                tc.tile_pool(name="mC", bufs=8) as mpool,
                tc.tile_pool(name="tlC", bufs=4) as tl,
                tc.tile_pool(name="paccC", bufs=1, space="PSUM") as pacc,
                tc.tile_pool(name="ptmpC", bufs=2, space="PSUM") as ptmp,
            ):
                GW2 = 2 * D + D + 1             # 193
                t_glob = 0
                for w in range(nw):
                    ntw = tiles_w[w]
                    p_all = pacc.tile([128, GW2], F32, tag="p_all2")
                    for t in range(ntw):
                        Gt = gat.tile([128, WC], F32, tag="G2")
                        nc.gpsimd.indirect_dma_start(
                            out=Gt[:], out_offset=None, in_=gathC[:],
                            in_offset=bass.IndirectOffsetOnAxis(
                                ap=S["idx_row"][:, t_glob:t_glob + 1], axis=0))
                        sbt = gat.tile([128, 1], F32, tag="sb2")
                        nc.gpsimd.indirect_dma_start(
                            out=sbt[:], out_offset=None, in_=a2dst[:],
                            in_offset=bass.IndirectOffsetOnAxis(
                                ap=S["idx_colL"][:, t_glob:t_glob + 1], axis=0))
                        z2 = gat.tile([128, 1], F32, tag="z2")
                        nc.vector.tensor_tensor(
                            out=z2[:], in0=Gt[:, WC - 1:WC], in1=sbt[:],
                            op=ALU.add)
                        z2s = gat.tile([128, 1], F32, tag="z2s")
                        nc.vector.tensor_scalar(out=z2s[:], in0=z2[:],
                                                scalar1=NEG_SLOPE, scalar2=None,
                                                op0=ALU.mult)
                        nc.vector.tensor_tensor(out=z2[:], in0=z2[:], in1=z2s[:],
                                                op=ALU.max)
                        e2 = gat.tile([128, 1], F32, tag="E2")
                        nc.scalar.activation(out=e2[:], in_=z2[:], func=AF.Exp)

                        cr = S["colrel"][:, t_glob:t_glob + 1]
                        st, sp = (t == 0), (t == ntw - 1)
                        M0 = mpool.tile([128, 128], F32, tag="M0")
                        nc.vector.tensor_scalar(
                            out=M0[:], in0=iota_f[:], scalar1=cr,
                            scalar2=None, op0=ALU.is_equal)
                        GA = mpool.tile([128, GW2], F32, tag="GA2")
                        nc.vector.tensor_scalar(
                            out=GA[:, 0:D], in0=Gt[:, 0:D],
                            scalar1=S["wnorm"][:, t_glob:t_glob + 1],
                            scalar2=None, op0=ALU.mult)
                        nc.vector.tensor_scalar(
                            out=GA[:, D:2 * D], in0=Gt[:, 2 * D + 1:3 * D + 1],
                            scalar1=S["wsage"][:, t_glob:t_glob + 1],
                            scalar2=None, op0=ALU.mult)
                        nc.vector.tensor_scalar(
                            out=GA[:, 2 * D:GW2], in0=Gt[:, D:2 * D + 1],
                            scalar1=e2[:, 0:1],
                            scalar2=None, op0=ALU.mult)
                        nc.tensor.matmul(out=p_all[:], lhsT=M0[:], rhs=GA[:],
                                         start=st, stop=sp)
                        t_glob += 1

                    # ---------- window tails ----------
                    # GCN2 (+w0, +w0*b2)
                    aggC = tl.tile([128, D], F32, tag="aggC")
                    nc.vector.tensor_copy(aggC[:], p_all[:, 0:D])
                    paT = ptmp.tile([D, 128], F32, tag="pt")
                    nc.tensor.matmul(out=paT[:], lhsT=aggC[:], rhs=ident[:],
                                     is_transpose=True)
                    aggT = tl.tile([D, 128], F32, tag="aggT")
                    nc.vector.tensor_copy(aggT[:], paT[:])
                    poT = ptmp.tile([D, 128], F32, tag="pt")
                    nc.tensor.matmul(out=poT[:], lhsT=W["gcn_w2"][:],
                                     rhs=aggT[:])
                    oTs = tl.tile([D, 128], F32, tag="oTs")
                    nc.scalar.activation(out=oTs[:], in_=poT[:],
                                         func=AF.Identity,
                                         scale=w64[:, 0:1], bias=b2w0[:, :1])
                    oTv = tl.tile([D, 128], F32, tag="oTv")
                    nc.vector.tensor_copy(oTv[:], oTs[:])
                    po = ptmp.tile([128, D], F32, tag="pt")
                    nc.tensor.matmul(out=po[:], lhsT=oTv[:], rhs=ident[:D, :D],
                                     is_transpose=True)
                    ogcn = tl.tile([128, D], F32, tag="ogcn")
                    nc.vector.tensor_copy(ogcn[:], po[:])

                    # GAT2 (+w1)
                    rd = tl.tile([128, 1], F32, tag="rd")
                    nc.vector.reciprocal(rd[:], p_all[:, 3 * D:3 * D + 1])
                    ogat = tl.tile([128, D], F32, tag="ogat")
                    nc.vector.tensor_scalar(out=ogat[:],
                                            in0=p_all[:, 2 * D:3 * D],
                                            scalar1=rd[:, :1],
                                            scalar2=wc[:, 1:2],
                                            op0=ALU.mult, op1=ALU.mult)

                    # SAGE2 (+w2); root rows come from the SBUF staging
                    meanC = tl.tile([128, D], F32, tag="meanC")
                    nc.vector.tensor_copy(meanC[:], p_all[:, D:2 * D])
                    pmT = ptmp.tile([D, 128], F32, tag="pt")
                    nc.tensor.matmul(out=pmT[:], lhsT=meanC[:], rhs=ident[:],
                                     is_transpose=True)
                    meanT = tl.tile([D, 128], F32, tag="meanT")
                    nc.vector.tensor_copy(meanT[:], pmT[:])
                    phdT = ptmp.tile([D, 128], F32, tag="pt")
                    nc.tensor.matmul(out=phdT[:],
                                     lhsT=st_hs[:, w * D:(w + 1) * D],
                                     rhs=ident[:], is_transpose=True)
                    hdT = tl.tile([D, 128], F32, tag="hdT")
                    nc.vector.tensor_copy(hdT[:], phdT[:])
                    psT = ptmp.tile([D, 128], F32, tag="pt")
                    nc.tensor.matmul(out=psT[:], lhsT=W["sage_wl2"][:],
                                     rhs=meanT[:], start=True, stop=False)
                    nc.tensor.matmul(out=psT[:], lhsT=W["sage_wr2"][:],
                                     rhs=hdT[:], start=False, stop=True)
                    sTs = tl.tile([D, 128], F32, tag="sTs")
                    nc.scalar.activation(out=sTs[:], in_=psT[:],
                                         func=AF.Identity,
                                         bias=W["sage_bl2c"][:, :1])
                    sTv = tl.tile([D, 128], F32, tag="sTv")
                    nc.vector.tensor_copy(sTv[:], sTs[:])
                    ps_ = ptmp.tile([128, D], F32, tag="pt")
                    nc.tensor.matmul(out=ps_[:], lhsT=sTv[:], rhs=ident[:D, :D],
                                     is_transpose=True)
                    s_sb = tl.tile([128, D], F32, tag="s_sb")
                    nc.vector.tensor_copy(s_sb[:], ps_[:])
                    sq = tl.tile([128, D], F32, tag="sq")
                    nc.vector.tensor_tensor(out=sq[:], in0=s_sb[:], in1=s_sb[:],
                                            op=ALU.mult)
                    ssum = tl.tile([128, 1], F32, tag="ssum")
                    nc.vector.tensor_reduce(out=ssum[:], in_=sq[:],
                                            axis=mybir.AxisListType.X,
                                            op=ALU.add)
                    nc.vector.tensor_scalar(out=ssum[:], in0=ssum[:],
                                            scalar1=1e-24, scalar2=None,
                                            op0=ALU.add)
                    rs = tl.tile([128, 1], F32, tag="rs")
                    nc.vector.reciprocal(rs[:], ssum[:])
                    rq = tl.tile([128, 1], F32, tag="rq")
                    nc.scalar.activation(out=rq[:], in_=rs[:], func=AF.Sqrt)
                    osage = tl.tile([128, D], F32, tag="osage")
                    nc.vector.tensor_scalar(out=osage[:], in0=s_sb[:],
                                            scalar1=rq[:, :1],
                                            scalar2=wc[:, 2:3],
                                            op0=ALU.mult, op1=ALU.mult)

                    # mix
                    mx1 = tl.tile([128, D], F32, tag="mx1")
                    nc.vector.tensor_tensor(out=mx1[:], in0=ogcn[:],
                                            in1=ogat[:], op=ALU.add)
                    mx2 = tl.tile([128, D], F32, tag="mx2")
                    nc.vector.tensor_tensor(out=mx2[:], in0=mx1[:],
                                            in1=osage[:], op=ALU.add)
                    nc.vector.tensor_tensor(out=st_out[:, w * D:(w + 1) * D],
                                            in0=mx2[:], in1=bgat[:],
                                            op=ALU.add)

            # ---- final output DMA: full windows, then the partial tail
            out_ap = bass.AP(out, 0, [[D, 128], [128 * D, nw - 1], [1, D]])
            nc.sync.dma_start(
                out=out_ap,
                in_=st_out[:, 0:(nw - 1) * D].rearrange(
                    "p (w c) -> p w c", w=nw - 1))
            nc.sync.dma_start(
                out=out[(nw - 1) * 128:shard, :],
                in_=st_out[0:rem, (nw - 1) * D:nw * D])
    return nc


# ---------------------------------------------------------------- host logic
DEBUG = {}
_PROG_CACHE = {}


def _run(nc, in_maps, trace=False):
    import time as _time
    if not nc.is_finalized():
        nc.finalize()
        # the jitted wrapper re-serializes the (frozen) module on every
        # call (~0.3s for this program); memoize on the instance
        _bir = nc.to_json_bytes()
        nc.to_json_bytes = lambda _b=_bir: _b
    t0 = _time.perf_counter()
    res = run_bass_kernel_spmd(nc, in_maps, list(range(NC_N)), trace=trace)
    DEBUG.setdefault("run_walls", []).append(_time.perf_counter() - t0)
    if res.exec_time_ns:
        DEBUG.setdefault("exec_ns", []).append(res.exec_time_ns)
    return res.results


def gnn_forward(x, edge_index, gate_w1, gate_b1, gate_w2, gate_b2,
                gcn_w1, gcn_b1, bn_gamma, bn_beta, gcn_w2, gcn_b2,
                gat_w1, gat_att_src1, gat_att_dst1, gat_b1,
                gat_w2, gat_att_src2, gat_att_dst2, gat_b2,
                sage_wl1, sage_bl1, sage_wr1, sage_wl2, sage_bl2, sage_wr2,
                trace=False):
    n_nodes = x.shape[0]
    x = np.asarray(x, np.float32)
    streams, tiles_w, T, shard, nw = build_schedule(
        np.asarray(edge_index), n_nodes)
    npad = nw * 128

    # ---- host weight folding (weights only, no data)
    w1r = np.asarray(gat_w1, np.float32).reshape(D, H1, D)
    vsrc = np.einsum("chj,hj->ch", w1r, np.asarray(gat_att_src1, np.float32))
    vdst = np.einsum("chj,hj->ch", w1r, np.asarray(gat_att_dst1, np.float32))
    vcat = np.concatenate([vsrc, vdst], axis=1).astype(np.float32)  # [64,8]
    v2 = (np.asarray(gat_w2, np.float32) @
          np.asarray(gat_att_src2, np.float32)[0])  # [256]
    u2 = (np.asarray(gat_w2, np.float32) @
          np.asarray(gat_att_dst2, np.float32)[0])
    v2u2 = np.stack([v2[:128], u2[:128], v2[128:], u2[128:]],
                    axis=1).astype(np.float32)  # [128,4]
    bn_s = (np.asarray(bn_gamma, np.float32) /
            np.sqrt(np.float32(1.0 + BN_EPS)))
    gcn1_s = bn_s.reshape(D, 1).astype(np.float32)
    gcn1_b = (bn_s * np.asarray(gcn_b1, np.float32) +
              np.asarray(bn_beta, np.float32)).reshape(D, 1).astype(np.float32)

    ck = (n_nodes, T, tuple(tiles_w))
    if ck in _PROG_CACHE:
        nc = _PROG_CACHE[ck]
    else:
        nc = build_full(n_nodes, tiles_w, T)
        _PROG_CACHE[ck] = nc

    common = {
        "vcat": vcat,
        "gw1": np.asarray(gate_w1, np.float32),
        "gb1": np.asarray(gate_b1, np.float32).reshape(1, D),
        "gw2": np.asarray(gate_w2, np.float32),
        "gb2": np.asarray(gate_b2, np.float32).reshape(1, 3),
        "gcn_w1": np.asarray(gcn_w1, np.float32),
        "gcn1_s": gcn1_s, "gcn1_b": gcn1_b,
        "sage_wl1": np.asarray(sage_wl1, np.float32),
        "sage_wr1": np.asarray(sage_wr1, np.float32),
        "sage_bl1": np.asarray(sage_bl1, np.float32).reshape(D, 1),
        "w2A": np.asarray(gat_w2, np.float32)[:128],
        "w2B": np.asarray(gat_w2, np.float32)[128:],
        "v2u2": v2u2,
        "w1h": np.asarray(gat_w1, np.float32),
        "b1c": np.asarray(gat_b1, np.float32).reshape(2, 128).T.copy(),
        "gcn_w2": np.asarray(gcn_w2, np.float32),
        "gcn_b2c": np.asarray(gcn_b2, np.float32).reshape(D, 1),
        "sage_wl2": np.asarray(sage_wl2, np.float32),
        "sage_wr2": np.asarray(sage_wr2, np.float32),
        "sage_bl2c": np.asarray(sage_bl2, np.float32).reshape(D, 1),
        "gat_b2r": np.asarray(gat_b2, np.float32).reshape(1, D),
    }
    in_maps = []
    for k in range(NC_N):
        m = dict(common)
        m.update(streams[k])
        xs65 = np.zeros((npad, D + 1), np.float16)
        xs65[:shard, :D] = x[k * shard:(k + 1) * shard]
        xs65[:shard, D] = 1.0
        m["xs65"] = xs65
        in_maps.append(m)
    res = _run(nc, in_maps, trace=trace)
    out = np.concatenate([res[k]["out"] for k in range(NC_N)], 0)
    return out.astype(np.float32)


def kernel(**inputs):
    return gnn_forward(**inputs)


# revision 31
# speedup vs baseline: 1.8571x; 1.2212x over previous
"""AdaptiveGNN (GCN+GAT+SAGE mixture) on 8 Trainium2 NeuronCores — single
NEFF launch with on-device AllGather collectives.

Strategy (destination-sharded graph parallelism, one program):
 - Core k owns nodes [k*6250, (k+1)*6250); edges (plus self-loops) are
   sorted by destination on the host into a static per-window tile
   schedule shared by all 8 cores (window = 128 destination rows).
 - Phase A (local): GAT attention projections a1 = x @ [v_src|v_dst] and
   column sums of x. Writes [x | 1 | a_src] rows plus a trailing
   column-sum row into a per-core DRAM buffer.
 - AllGather #1 ([6273, 69] per core -> [50184, 69]): every core now has
   the full graph's source features + attention sources (halo exchange).
 - Gate MLP computed redundantly on every core from the 8 column-sum rows.
 - Phase B: layer 1 of all three branches for the local destination
   shard. Per edge-tile: indirect-DMA gather of source rows from the
   AllGathered buffer, local gather of a_dst rows, one-hot "selection"
   matmuls accumulate segment sums in PSUM. Window tails produce
   h1 = relu(bn(gcn1)), h2 = elu(gat1) @ W2 (+ attn scalars), hs = sage1;
   all written into the second per-core DRAM buffer [h1|h2|1|hs|a2src].
 - AllGather #2 ([6273, 194] per core -> [50184, 194]).
 - Phase C: layer 2 of all three branches + gated mix -> final rows.
 - Host sends only the x shard, edge streams and weights (~4MB/core)
   and receives the per-core output rows; one PJRT dispatch total.
"""

import hashlib
import os
import sys

sys.path.insert(0, "/opt/trn_rl_repo")

# Persistent XLA compilation cache: run_bass_via_pjrt re-jits a fresh
# closure every call, so jax's in-memory caches (weakref-keyed) can never
# hit; the disk cache is keyed on HLO bytes and skips the whole
# walrus+neuronxcc+load pipeline on repeat calls.
os.environ.setdefault("JAX_COMPILATION_CACHE_DIR",
                      os.path.expanduser("~/.cache/jax_bass_cache"))
os.environ.setdefault("JAX_PERSISTENT_CACHE_MIN_COMPILE_TIME_SECS", "0")
os.environ.setdefault("JAX_PERSISTENT_CACHE_MIN_ENTRY_SIZE_BYTES", "0")

import numpy as np

from concourse import bacc, bass, mybir, tile
from concourse import bass2jax as _b2j
from concourse.bass_utils import run_bass_kernel_spmd
import concourse.tile_sem_assignment as _tsa

# Memoize the bass_exec HLO -> NEFF-wrapped-HLO compile step. The jitted
# wrapper is rebuilt per call, so XLA recompiles the HLO each time; without
# this the BIR verify/walrus subprocess (~1.4s for this program) runs on
# every kernel() invocation. The serialized HloModuleProto differs across
# traces only in its module-id counter, so zero it for the cache key.
_HOOK_CACHE: dict = {}
_orig_neuronx_cc_hook = _b2j.neuronx_cc_hook


def _hook_key(code):
    import libneuronxla.proto.hlo_pb2 as _hlo_pb2
    p = _hlo_pb2.HloModuleProto.FromString(bytes(code))
    p.id = 0                           # per-trace module counter
    p.ClearField("stack_frame_index")  # caller source lines
    return hashlib.sha256(p.SerializeToString()).digest()


def _caching_neuronx_cc_hook(code, code_format, platform_version, file_prefix):
    if b"bass_exec" not in code:
        return _orig_neuronx_cc_hook(code, code_format, platform_version,
                                     file_prefix)
    try:
        key = _hook_key(code)
    except Exception:
        key = hashlib.sha256(bytes(code)).digest()
    r = _HOOK_CACHE.get(key)
    if r is None:
        r = _orig_neuronx_cc_hook(code, code_format, platform_version,
                                  file_prefix)
        _HOOK_CACHE[key] = r
    return r


_b2j.neuronx_cc_hook = _caching_neuronx_cc_hook

# Reuse the jitted PJRT callable across calls. run_bass_via_pjrt builds a
# fresh closure + jax.jit per invocation, so jax's jit cache misses and the
# executable is re-created and the NEFF re-loaded onto all 8 cores every
# call. This is a faithful fork of its multi-core path with the jitted
# function memoized per finalized program; run_bass_kernel_spmd still
# drives it (falls back to the stock path on any surprise).
_PJRT_CACHE: dict = {}
_orig_run_bass_via_pjrt = _b2j.run_bass_via_pjrt


def _build_pjrt_callable(nc, n_cores):
    import jax
    from jax.experimental.shard_map import shard_map
    from jax.sharding import Mesh, PartitionSpec

    _b2j.install_neuronx_cc_hook()
    assert nc.dbg_addr is None and not nc.dbg_callbacks
    partition_name = (nc.partition_id_tensor.name
                      if nc.partition_id_tensor else None)
    in_names, out_names, out_avals = [], [], []
    for alloc in nc.m.functions[0].allocations:
        if not isinstance(alloc, mybir.MemoryLocationSet):
            continue
        name = alloc.memorylocations[0].name
        if alloc.kind == "ExternalInput":
            if name != partition_name:
                in_names.append(name)
        elif alloc.kind == "ExternalOutput":
            out_names.append(name)
            out_avals.append(jax.core.ShapedArray(
                tuple(alloc.tensor_shape), mybir.dt.np(alloc.dtype)))
    n_params = len(in_names)
    n_outs = len(out_avals)
    all_in_names = list(in_names) + list(out_names)
    if partition_name is not None:
        all_in_names.append(partition_name)
    donate = tuple(range(n_params, n_params + n_outs))

    def _body(*args):
        operands = list(args)
        if partition_name is not None:
            operands.append(_b2j.partition_id_tensor())
        return tuple(_b2j._bass_exec_p.bind(
            *operands,
            out_avals=tuple(out_avals),
            in_names=tuple(all_in_names),
            out_names=tuple(out_names),
            lowering_input_output_aliases=(),
            sim_require_finite=True,
            sim_require_nnan=True,
            nc=nc,
        ))

    devices = jax.devices()[:n_cores]
    assert len(devices) == n_cores
    mesh = Mesh(np.asarray(devices), ("core",))
    in_specs = (PartitionSpec("core"),) * (n_params + n_outs)
    out_specs = (PartitionSpec("core"),) * n_outs
    sharded = jax.jit(
        shard_map(_body, mesh=mesh, in_specs=in_specs, out_specs=out_specs,
                  check_rep=False),
        donate_argnums=donate, keep_unused=True)

    # Donated output buffers are created device-side (our program writes
    # every output element, so their contents never matter) — uploading
    # host zeros would cost another ~6MB of tunnel bytes per call.
    import jax.numpy as jnp
    from jax.sharding import NamedSharding
    zero_shardings = tuple(NamedSharding(mesh, PartitionSpec("core"))
                           for _ in out_avals)
    zeros_maker = jax.jit(
        lambda: tuple(jnp.zeros((n_cores * a.shape[0], *a.shape[1:]), a.dtype)
                      for a in out_avals),
        out_shardings=zero_shardings)

    def call(in_maps):
        per_core = [[np.asarray(m[name]) for name in in_names]
                    for m in in_maps]
        concat_in = [
            np.concatenate([per_core[c][i] for c in range(n_cores)], axis=0)
            for i in range(n_params)]
        concat_zeros = zeros_maker()
        out_arrs = sharded(*concat_in, *concat_zeros)
        return [
            {name: np.asarray(out_arrs[i]).reshape(
                n_cores, *out_avals[i].shape)[c]
             for i, name in enumerate(out_names)}
            for c in range(n_cores)]

    return call


def _cached_run_bass_via_pjrt(nc, in_maps, n_cores):
    try:
        ent = _PJRT_CACHE.get(id(nc))
        if ent is None:
            # hold the nc ref in the entry so its id() stays unique
            ent = (_build_pjrt_callable(nc, n_cores), nc)
            _PJRT_CACHE[id(nc)] = ent
        return ent[0](in_maps)
    except Exception:
        return _orig_run_bass_via_pjrt(nc, in_maps, n_cores)


_b2j.run_bass_via_pjrt = _cached_run_bass_via_pjrt

# Clamp Tile's DMA-completion semaphore lanes (kernel-tail Drain waits on
# every producer semaphore; walrus rejects instructions with too many
# sync waits).
_tsa.NUM_HWDGE_SEMS = 8
_tsa.NUM_SWDGE_GLOBAL_SEMS = 8

F32 = mybir.dt.float32
F16 = mybir.dt.float16
I32 = mybir.dt.int32
AF = mybir.ActivationFunctionType
ALU = mybir.AluOpType

NC_N = 8          # cores
D = 64            # feature dim
H1 = 4            # GAT hidden heads
WB = D + 1 + H1   # phase-B gather row: [x | 1 | a_src]            = 69
WC = 3 * D + 2    # phase-C gather row: [h1 | h2 | 1 | hs | a2src] = 194
NEG_SLOPE = 0.2
BN_EPS = 1e-5


# ----------------------------------------------------------------- host prep
def build_schedule(edge_index, n_nodes):
    """Sort edges (plus self-loops) by destination, shard by destination,
    and produce a tile schedule common to all cores plus per-core streams.
    Source indices are remapped into the AllGather row space
    (node n -> (n // shard) * (npad + 1) + n % shard)."""
    shard = n_nodes // NC_N
    nw = (shard + 127) // 128
    # per-core AllGather rows: +1 csum row, +1 pad so the collective's
    # element count stays even (NRT needs 8-byte-aligned collective sizes)
    nprow = nw * 128 + 2
    row = edge_index[0].astype(np.int64)
    col = edge_index[1].astype(np.int64)
    loops = np.arange(n_nodes, dtype=np.int64)
    r_all = np.concatenate([row, loops])
    c_all = np.concatenate([col, loops])

    # GCN symmetric normalization (self-loops included)
    deg = np.bincount(c_all, minlength=n_nodes).astype(np.float64)
    dis = np.where(deg > 0, deg ** -0.5, 0.0)
    wnorm_all = (dis[r_all] * dis[c_all]).astype(np.float32)
    # SAGE mean weights (real edges only; zero on appended self-loops)
    cnt = np.bincount(col, minlength=n_nodes).astype(np.float64)
    ws = (1.0 / np.maximum(cnt, 1.0))[col].astype(np.float32)
    wsage_all = np.concatenate([ws, np.zeros(n_nodes, np.float32)])
    # source node id -> AllGathered row
    rg_all = ((r_all // shard) * nprow + (r_all % shard)).astype(np.int64)

    per_core = []
    counts = np.zeros((NC_N, nw), dtype=np.int64)
    for k in range(NC_N):
        lo, hi = k * shard, (k + 1) * shard
        sel = np.nonzero((c_all >= lo) & (c_all < hi))[0]
        cl = c_all[sel] - lo
        order = np.argsort(cl, kind="stable")
        sel = sel[order]
        cl = cl[order]
        w_of = cl // 128
        cnts = np.bincount(w_of, minlength=nw)
        counts[k] = cnts
        per_core.append((sel, cl, cnts))

    tiles_w = np.maximum(1, (counts.max(axis=0) + 127) // 128)
    T = int(tiles_w.sum())

    streams = []
    for k in range(NC_N):
        sel, cl, cnts = per_core[k]
        idx_row = np.zeros(T * 128, np.int32)
        idx_colL = np.zeros(T * 128, np.int32)
        colrel = np.full(T * 128, -1.0, np.float32)
        wnorm = np.zeros(T * 128, np.float32)
        wsage = np.zeros(T * 128, np.float32)
        pos = 0      # position in padded stream
        epos = 0     # position in this core's sorted edge list
        for w in range(nw):
            cw = int(cnts[w])
            seg = sel[epos:epos + cw]
            base = pos
            idx_row[base:base + cw] = rg_all[seg]
            idx_colL[base:base + cw] = cl[epos:epos + cw]
            colrel[base:base + cw] = (cl[epos:epos + cw] % 128).astype(np.float32)
            wnorm[base:base + cw] = wnorm_all[seg]
            wsage[base:base + cw] = wsage_all[seg]
            epos += cw
            pos += int(tiles_w[w]) * 128
        # idx_row needs 16 bits (max 8*nprow), idx_colL 15 -> one int32
        packed = idx_row | (idx_colL << 16)
        st = {
            "idx_pack": packed.reshape(T, 128).T.copy(),
            "colrel": colrel.reshape(T, 128).T.astype(np.float16),
            "wnorm": wnorm.reshape(T, 128).T.astype(np.float16),
            "wsage": wsage.reshape(T, 128).T.astype(np.float16),
        }
        streams.append(st)
    return streams, [int(t) for t in tiles_w], T, shard, nw


# ------------------------------------------------------------- device pieces
def _load_w(nc, pool, dram, shape, tag, in_dtype=None):
    ld = pool.tile(list(shape), in_dtype or F32, tag=tag + "_ld")
    nc.sync.dma_start(out=ld[:], in_=dram[:])
    t = pool.tile(list(shape), F32, tag=tag)
    nc.vector.tensor_copy(t[:], ld[:])
    return t


# ----------------------------------------------------------------- the build
def build_full(n_nodes, tiles_w, T):
    shard = n_nodes // NC_N
    nw = (shard + 127) // 128
    npad = nw * 128
    nprow = npad + 2   # +1 csum row, +1 pad row (8-byte collective align)
    gfull = NC_N * nprow
    rem = shard - (nw - 1) * 128       # rows in last output window

    nc = bacc.Bacc()
    dr = {}
    for nm, shp, dt in [
        ("xs65", [npad, D + 1], F16),
        ("idx_pack", [128, T], I32),
        ("colrel", [128, T], F16), ("wnorm", [128, T], F16),
        ("wsage", [128, T], F16),
        ("vcat", [D, 2 * H1], F32),
        ("gw1", [D, D], F32), ("gb1", [1, D], F32),
        ("gw2", [D, 3], F32), ("gb2", [1, 3], F32),
        ("gcn_w1", [D, D], F32), ("gcn1_s", [D, 1], F32), ("gcn1_b", [D, 1], F32),
        ("sage_wl1", [D, D], F32), ("sage_wr1", [D, D], F32),
        ("sage_bl1", [D, 1], F32),
        ("w2A", [128, D], F32), ("w2B", [128, D], F32),
        ("v2u2", [128, 4], F32), ("w1h", [D, 4 * D], F32),
        ("b1c", [128, 2], F32),
        ("gcn_w2", [D, D], F32), ("gcn_b2c", [D, 1], F32),
        ("sage_wl2", [D, D], F32), ("sage_wr2", [D, D], F32),
        ("sage_bl2c", [D, 1], F32), ("gat_b2r", [1, D], F32),
    ]:
        dr[nm] = nc.dram_tensor(nm, shp, dt, kind="ExternalInput")
    out = nc.dram_tensor("out", [shard, D], F16, kind="ExternalOutput")
    c_ident = nc.inline_tensor(np.eye(128, dtype=np.float32), name="cident")
    c_iota = nc.inline_tensor(
        np.tile(np.arange(128, dtype=np.float32), (128, 1)), name="ciota")

    with tile.TileContext(nc) as tc:
        with (
            tc.tile_pool(name="const", bufs=1) as const,
            tc.tile_pool(name="wts", bufs=1) as wts,
            tc.tile_pool(name="stream", bufs=1) as stream,
            tc.tile_pool(name="stage", bufs=1) as stage,
            tc.tile_pool(name="dramp", bufs=1, space="DRAM") as dramp,
        ):
            ident = _load_w(nc, const, c_ident, (128, 128), "ident")
            iota_f = _load_w(nc, const, c_iota, (128, 128), "iota_f")
            ones_col = const.tile([128, 1], F32, tag="ones_col")
            nc.vector.memset(ones_col[:], 1.0)
            ones_row = const.tile([1, 128], F32, tag="ones_row")
            nc.vector.memset(ones_row[:], 1.0)

            # DRAM buffers for the halo exchange (AllGather outputs live in
            # the Shared scratchpad: NRT's fast HBM-HBM collective path)
            bufB = dramp.tile([nprow, WB], F32, tag="bufB")
            gathB = nc.dram_tensor("gathB_sh", [gfull, WB], F32,
                                   addr_space="Shared")
            bufC = dramp.tile([nprow, WC], F32, tag="bufC")
            gathC = nc.dram_tensor("gathC_sh", [gfull, WC], F32,
                                   addr_space="Shared")
            adst = dramp.tile([npad, H1], F32, tag="adst")
            a2dst = dramp.tile([npad, 1], F32, tag="a2dst")

            # ---- weights to SBUF
            W = {}
            for nm, shp in [
                ("vcat", (D, 2 * H1)),
                ("gw1", (D, D)), ("gb1", (1, D)), ("gw2", (D, 3)), ("gb2", (1, 3)),
                ("gcn_w1", (D, D)), ("gcn1_s", (D, 1)), ("gcn1_b", (D, 1)),
                ("sage_wl1", (D, D)), ("sage_wr1", (D, D)), ("sage_bl1", (D, 1)),
                ("w2A", (128, D)), ("w2B", (128, D)), ("v2u2", (128, 4)),
                ("w1h", (D, 4 * D)), ("b1c", (128, 2)),
                ("gcn_w2", (D, D)), ("gcn_b2c", (D, 1)),
                ("sage_wl2", (D, D)), ("sage_wr2", (D, D)),
                ("sage_bl2c", (D, 1)), ("gat_b2r", (1, D)),
            ]:
                W[nm] = _load_w(nc, wts, dr[nm], shp, nm)

            # ---- streams to SBUF (f16/packed-i32 halve the tunnel bytes)
            S = {}
            pk = stream.tile([128, T], I32, tag="idx_pack")
            nc.sync.dma_start(out=pk[:], in_=dr["idx_pack"][:])
            t = stream.tile([128, T], I32, tag="idx_row")
            nc.vector.tensor_scalar(out=t[:], in0=pk[:], scalar1=0xFFFF,
                                    scalar2=None, op0=ALU.bitwise_and)
            S["idx_row"] = t
            t = stream.tile([128, T], I32, tag="idx_colL")
            nc.vector.tensor_scalar(out=t[:], in0=pk[:], scalar1=16,
                                    scalar2=None,
                                    op0=ALU.logical_shift_right)
            S["idx_colL"] = t
            for nm in ("colrel", "wnorm", "wsage"):
                raw = stream.tile([128, T], F16, tag=nm + "_raw")
                nc.sync.dma_start(out=raw[:], in_=dr[nm][:])
                t = stream.tile([128, T], F32, tag=nm)
                nc.vector.tensor_copy(t[:], raw[:])
                S[nm] = t

            # ---- staging buffers (SBUF-resident across phases)
            st_hs = stage.tile([128, nw * D], F32, tag="st_hs")
            st_out = stage.tile([128, nw * D], F16, tag="st_out")

            # ================================================== phase A
            with (
                tc.tile_pool(name="sbA", bufs=3) as sbA,
                tc.tile_pool(name="psA", bufs=2, space="PSUM") as psA,
                tc.tile_pool(name="pcsA", bufs=1, space="PSUM") as pcsA,
            ):
                csum_p = pcsA.tile([1, D], F32, tag="csum")
                for w in range(nw):
                    xt0 = sbA.tile([128, D + 1], F16, tag="xt0")
                    nc.sync.dma_start(out=xt0[:],
                                      in_=dr["xs65"][w * 128:(w + 1) * 128, :])
                    xt = sbA.tile([128, D + 1], F32, tag="xt")
                    nc.vector.tensor_copy(xt[:], xt0[:])
                    nc.sync.dma_start(
                        out=bufB[w * 128:(w + 1) * 128, 0:D + 1], in_=xt[:])
                    pT = psA.tile([D, 128], F32, tag="pT")
                    nc.tensor.matmul(out=pT[:], lhsT=xt[:, 0:D], rhs=ident[:],
                                     is_transpose=True)
                    xT = sbA.tile([D, 128], F32, tag="xT")
                    nc.vector.tensor_copy(xT[:], pT[:])
                    pa = psA.tile([2 * H1, 128], F32, tag="pa")
                    nc.tensor.matmul(out=pa[:], lhsT=W["vcat"][:], rhs=xT[:])
                    aT = sbA.tile([2 * H1, 128], F32, tag="aT")
                    nc.vector.tensor_copy(aT[:], pa[:])
                    pb = psA.tile([128, 2 * H1], F32, tag="pb")
                    nc.tensor.matmul(out=pb[:], lhsT=aT[:],
                                     rhs=ident[:2 * H1, :2 * H1],
                                     is_transpose=True)
                    ab = sbA.tile([128, 2 * H1], F32, tag="ab")
                    nc.vector.tensor_copy(ab[:], pb[:])
                    nc.sync.dma_start(
                        out=bufB[w * 128:(w + 1) * 128, D + 1:WB],
                        in_=ab[:, 0:H1])
                    nc.sync.dma_start(
                        out=adst[w * 128:(w + 1) * 128, :], in_=ab[:, H1:2 * H1])
                    nc.tensor.matmul(out=csum_p[:], lhsT=ones_col[:],
                                     rhs=xt[:, 0:D],
                                     start=(w == 0), stop=(w == nw - 1))
                cs = sbA.tile([1, D], F32, tag="cs")
                nc.vector.tensor_copy(cs[:], csum_p[:])
                nc.sync.dma_start(out=bufB[npad:npad + 1, 0:D], in_=cs[:])

            # ============================================== AllGather #1
            nc.gpsimd.collective_compute(
                "AllGather", ALU.bypass,
                replica_groups=[list(range(NC_N))],
                ins=[bufB[:].opt()], outs=[gathB[:].opt()])

            # ================================================== phase B
            with (
                tc.tile_pool(name="gatB", bufs=8) as gat,
                tc.tile_pool(name="mB", bufs=8) as mpool,
                tc.tile_pool(name="smB", bufs=3) as sm,
                tc.tile_pool(name="tlB", bufs=4) as tl,
                tc.tile_pool(name="paccB", bufs=1, space="PSUM") as pacc,
                tc.tile_pool(name="ptmpB", bufs=2, space="PSUM") as ptmp,
            ):
                # ---- gate MLP from the 8 AllGathered csum rows
                cs8l = sm.tile([NC_N, D], F32, tag="g_cs8l")
                for k in range(NC_N):
                    nc.sync.dma_start(
                        out=cs8l[k:k + 1, :],
                        in_=gathB[k * nprow + npad:k * nprow + npad + 1, 0:D])
                cs8 = sm.tile([NC_N, D], F32, tag="g_cs8")
                nc.vector.tensor_copy(cs8[:], cs8l[:])
                pxb = ptmp.tile([1, D], F32, tag="pt")
                nc.tensor.matmul(out=pxb[:], lhsT=ones_col[:NC_N, :1],
                                 rhs=cs8[:])
                xbar = sm.tile([1, D], F32, tag="g_xbar")
                nc.vector.tensor_scalar(out=xbar[:], in0=pxb[:],
                                        scalar1=1.0 / n_nodes, scalar2=None,
                                        op0=ALU.mult)
                pxT = ptmp.tile([D, 1], F32, tag="pt")
                nc.tensor.matmul(out=pxT[:], lhsT=xbar[:], rhs=ident[:1, :1],
                                 is_transpose=True)
                xbT = sm.tile([D, 1], F32, tag="g_xbT")
                nc.vector.tensor_copy(xbT[:], pxT[:])
                pg1 = ptmp.tile([1, D], F32, tag="pt")
                nc.tensor.matmul(out=pg1[:], lhsT=xbT[:], rhs=W["gw1"][:])
                g1 = sm.tile([1, D], F32, tag="g_g1")
                nc.vector.tensor_tensor(out=g1[:], in0=pg1[:], in1=W["gb1"][:],
                                        op=ALU.add)
                g1r = sm.tile([1, D], F32, tag="g_g1r")
                nc.vector.tensor_scalar(out=g1r[:], in0=g1[:], scalar1=0.0,
                                        scalar2=None, op0=ALU.max)
                pg1T = ptmp.tile([D, 1], F32, tag="pt")
                nc.tensor.matmul(out=pg1T[:], lhsT=g1r[:], rhs=ident[:1, :1],
                                 is_transpose=True)
                g1T = sm.tile([D, 1], F32, tag="g_g1T")
                nc.vector.tensor_copy(g1T[:], pg1T[:])
                pg2 = ptmp.tile([1, 3], F32, tag="pt")
                nc.tensor.matmul(out=pg2[:], lhsT=g1T[:], rhs=W["gw2"][:])
                g2 = sm.tile([1, 3], F32, tag="g_g2")
                nc.vector.tensor_tensor(out=g2[:], in0=pg2[:], in1=W["gb2"][:],
                                        op=ALU.add)
                g2e = sm.tile([1, 3], F32, tag="g_g2e")
                nc.scalar.activation(out=g2e[:], in_=g2[:], func=AF.Exp)
                g2s = sm.tile([1, 1], F32, tag="g_g2s")
                nc.vector.tensor_reduce(out=g2s[:], in_=g2e[:],
                                        axis=mybir.AxisListType.X, op=ALU.add)
                g2r = sm.tile([1, 1], F32, tag="g_g2r")
                nc.vector.reciprocal(g2r[:], g2s[:])
                gate_sb = sm.tile([1, 3], F32, tag="g_gate")
                nc.vector.tensor_scalar(out=gate_sb[:], in0=g2e[:],
                                        scalar1=g2r[:, :1], scalar2=None,
                                        op0=ALU.mult)
                # gate scalar broadcasts (used by phase C tails)
                pw128 = ptmp.tile([128, 3], F32, tag="pt")
                nc.tensor.matmul(out=pw128[:], lhsT=ones_row[:], rhs=gate_sb[:])
                wc = wts.tile([128, 3], F32, tag="wc")
                nc.vector.tensor_copy(wc[:], pw128[:])
                pw64 = ptmp.tile([D, 3], F32, tag="pt")
                nc.tensor.matmul(out=pw64[:], lhsT=ones_row[:1, :D],
                                 rhs=gate_sb[:])
                w64 = wts.tile([D, 3], F32, tag="w64")
                nc.vector.tensor_copy(w64[:], pw64[:])
                b2w0 = wts.tile([D, 1], F32, tag="b2w0")
                nc.vector.tensor_scalar(out=b2w0[:], in0=W["gcn_b2c"][:],
                                        scalar1=w64[:, 0:1], scalar2=None,
                                        op0=ALU.mult)
                pbg = ptmp.tile([128, D], F32, tag="pt")
                nc.tensor.matmul(out=pbg[:], lhsT=ones_row[:],
                                 rhs=W["gat_b2r"][:])
                bgat = wts.tile([128, D], F32, tag="bgat")
                nc.vector.tensor_scalar(out=bgat[:], in0=pbg[:],
                                        scalar1=wc[:, 1:2], scalar2=None,
                                        op0=ALU.mult)

                # ---- edge loop: one shared one-hot per tile, row-scaled rhs
                # p_all[dest, :] = sum_e onehot(dest)_e * [x*wn | x*ws | (x|1)*e_h ...]
                GAW = 2 * D + H1 * (D + 1)      # 388
                t_glob = 0
                for w in range(nw):
                    ntw = tiles_w[w]
                    p_all = pacc.tile([128, GAW], F32, tag="p_all")
                    for t in range(ntw):
                        Gt = gat.tile([128, WB], F32, tag="G")
                        nc.gpsimd.indirect_dma_start(
                            out=Gt[:], out_offset=None, in_=gathB[:],
                            in_offset=bass.IndirectOffsetOnAxis(
                                ap=S["idx_row"][:, t_glob:t_glob + 1], axis=0))
                        sbt = gat.tile([128, H1], F32, tag="sbt")
                        nc.gpsimd.indirect_dma_start(
                            out=sbt[:], out_offset=None, in_=adst[:],
                            in_offset=bass.IndirectOffsetOnAxis(
                                ap=S["idx_colL"][:, t_glob:t_glob + 1], axis=0))
                        zt = gat.tile([128, H1], F32, tag="z")
                        nc.vector.tensor_tensor(
                            out=zt[:], in0=Gt[:, D + 1:WB], in1=sbt[:],
                            op=ALU.add)
                        zs = gat.tile([128, H1], F32, tag="zs")
                        nc.vector.tensor_scalar(out=zs[:], in0=zt[:],
                                                scalar1=NEG_SLOPE, scalar2=None,
                                                op0=ALU.mult)
                        nc.vector.tensor_tensor(out=zt[:], in0=zt[:], in1=zs[:],
                                                op=ALU.max)
                        et = gat.tile([128, H1], F32, tag="E")
                        nc.scalar.activation(out=et[:], in_=zt[:], func=AF.Exp)

                        cr = S["colrel"][:, t_glob:t_glob + 1]
                        st, sp = (t == 0), (t == ntw - 1)
                        M0 = mpool.tile([128, 128], F32, tag="M0")
                        nc.vector.tensor_scalar(
                            out=M0[:], in0=iota_f[:], scalar1=cr,
                            scalar2=None, op0=ALU.is_equal)
                        GA = mpool.tile([128, GAW], F32, tag="GA")
                        nc.vector.tensor_scalar(
                            out=GA[:, 0:D], in0=Gt[:, 0:D],
                            scalar1=S["wnorm"][:, t_glob:t_glob + 1],
                            scalar2=None, op0=ALU.mult)
                        nc.vector.tensor_scalar(
                            out=GA[:, D:2 * D], in0=Gt[:, 0:D],
                            scalar1=S["wsage"][:, t_glob:t_glob + 1],
                            scalar2=None, op0=ALU.mult)
                        for h in range(H1):
                            nc.vector.tensor_scalar(
                                out=GA[:, 2 * D + h * (D + 1):
                                       2 * D + (h + 1) * (D + 1)],
                                in0=Gt[:, 0:D + 1],
                                scalar1=et[:, h:h + 1],
                                scalar2=None, op0=ALU.mult)
                        nc.tensor.matmul(out=p_all[:], lhsT=M0[:], rhs=GA[:],
                                         start=st, stop=sp)
                        t_glob += 1

                    # ---------- window tails ----------
                    rows = slice(w * 128, (w + 1) * 128)
                    # GCN1: h1 = relu(s*(W1^T aggT) + b)
                    aggC = tl.tile([128, D], F32, tag="aggC")
                    nc.vector.tensor_copy(aggC[:], p_all[:, 0:D])
                    paT = ptmp.tile([D, 128], F32, tag="pt")
                    nc.tensor.matmul(out=paT[:], lhsT=aggC[:], rhs=ident[:],
                                     is_transpose=True)
                    aggT = tl.tile([D, 128], F32, tag="aggT")
                    nc.vector.tensor_copy(aggT[:], paT[:])
                    ph1T = ptmp.tile([D, 128], F32, tag="pt")
                    nc.tensor.matmul(out=ph1T[:], lhsT=W["gcn_w1"][:],
                                     rhs=aggT[:])
                    h1Ts = tl.tile([D, 128], F32, tag="h1Ts")
                    nc.scalar.activation(out=h1Ts[:], in_=ph1T[:], func=AF.Relu,
                                         scale=W["gcn1_s"][:, :1],
                                         bias=W["gcn1_b"][:, :1])
                    h1Tv = tl.tile([D, 128], F32, tag="h1Tv")
                    nc.vector.tensor_copy(h1Tv[:], h1Ts[:])
                    ph1 = ptmp.tile([128, D], F32, tag="pt")
                    nc.tensor.matmul(out=ph1[:], lhsT=h1Tv[:], rhs=ident[:D, :D],
                                     is_transpose=True)
                    h1sb = tl.tile([128, D], F32, tag="h1sb")
                    nc.vector.tensor_copy(h1sb[:], ph1[:])
                    nc.sync.dma_start(out=bufC[rows, 0:D], in_=h1sb[:])

                    # GAT1 heads -> x2T halves -> h2, a2
                    x2TA = tl.tile([128, 128], F32, tag="x2TA")
                    x2TB = tl.tile([128, 128], F32, tag="x2TB")
                    for h in range(H1):
                        hb = 2 * D + h * (D + 1)
                        rd = tl.tile([128, 1], F32, tag="rd")
                        nc.vector.reciprocal(rd[:], p_all[:, hb + D:hb + D + 1])
                        hd_sb = tl.tile([128, D], F32, tag="hd_sb")
                        nc.vector.tensor_scalar(
                            out=hd_sb[:], in0=p_all[:, hb:hb + D],
                            scalar1=rd[:, :1], scalar2=None, op0=ALU.mult)
                        pht = ptmp.tile([D, 128], F32, tag="pt")
                        nc.tensor.matmul(out=pht[:], lhsT=hd_sb[:], rhs=ident[:],
                                         is_transpose=True)
                        hdT = tl.tile([D, 128], F32, tag="hdT_g")
                        nc.vector.tensor_copy(hdT[:], pht[:])
                        pxh = ptmp.tile([D, 128], F32, tag="pt")
                        nc.tensor.matmul(out=pxh[:],
                                         lhsT=W["w1h"][:, h * D:(h + 1) * D],
                                         rhs=hdT[:])
                        stgt = x2TA if h < 2 else x2TB
                        nc.vector.tensor_copy(
                            stgt[(h % 2) * D:(h % 2 + 1) * D, :], pxh[:])
                    x2T = []
                    for half, px in enumerate((x2TA, x2TB)):
                        yT = tl.tile([128, 128], F32, tag="yT")
                        nc.vector.tensor_scalar(
                            out=yT[:], in0=px[:],
                            scalar1=W["b1c"][:, half:half + 1], scalar2=None,
                            op0=ALU.add)
                        ymin = tl.tile([128, 128], F32, tag="ymin")
                        nc.vector.tensor_scalar(out=ymin[:], in0=yT[:],
                                                scalar1=0.0, scalar2=None,
                                                op0=ALU.min)
                        yexp = tl.tile([128, 128], F32, tag="yexp")
                        nc.scalar.activation(out=yexp[:], in_=ymin[:],
                                             func=AF.Exp)
                        ye1 = tl.tile([128, 128], F32, tag="ye1")
                        nc.vector.tensor_scalar(out=ye1[:], in0=yexp[:],
                                                scalar1=-1.0, scalar2=None,
                                                op0=ALU.add)
                        ymax = tl.tile([128, 128], F32, tag="ymax")
                        nc.vector.tensor_scalar(out=ymax[:], in0=yT[:],
                                                scalar1=0.0, scalar2=None,
                                                op0=ALU.max)
                        xt2 = tl.tile([128, 128], F32, tag=f"x2T{half}")
                        nc.vector.tensor_tensor(out=xt2[:], in0=ymax[:],
                                                in1=ye1[:], op=ALU.add)
                        x2T.append(xt2)
                    ph2T = ptmp.tile([D, 128], F32, tag="pt")
                    nc.tensor.matmul(out=ph2T[:], lhsT=W["w2A"][:],
                                     rhs=x2T[0][:], start=True, stop=False)
                    nc.tensor.matmul(out=ph2T[:], lhsT=W["w2B"][:],
                                     rhs=x2T[1][:], start=False, stop=True)
                    pa2T = ptmp.tile([2, 128], F32, tag="pt")
                    nc.tensor.matmul(out=pa2T[:], lhsT=W["v2u2"][:, 0:2],
                                     rhs=x2T[0][:], start=True, stop=False)
                    nc.tensor.matmul(out=pa2T[:], lhsT=W["v2u2"][:, 2:4],
                                     rhs=x2T[1][:], start=False, stop=True)
                    h2Ts = tl.tile([D, 128], F32, tag="h2Ts")
                    nc.vector.tensor_copy(h2Ts[:], ph2T[:])
                    ph2 = ptmp.tile([128, D], F32, tag="pt")
                    nc.tensor.matmul(out=ph2[:], lhsT=h2Ts[:], rhs=ident[:D, :D],
                                     is_transpose=True)
                    h2sb = tl.tile([128, D], F32, tag="h2sb")
                    nc.vector.tensor_copy(h2sb[:], ph2[:])
                    nc.sync.dma_start(out=bufC[rows, D:2 * D], in_=h2sb[:])
                    nc.sync.dma_start(out=bufC[rows, 2 * D:2 * D + 1],
                                      in_=ones_col[:])
                    a2Ts = tl.tile([2, 128], F32, tag="a2Ts")
                    nc.vector.tensor_copy(a2Ts[:], pa2T[:])
                    pa2 = ptmp.tile([128, 2], F32, tag="pt")
                    nc.tensor.matmul(out=pa2[:], lhsT=a2Ts[:], rhs=ident[:2, :2],
                                     is_transpose=True)
                    a2sb = tl.tile([128, 2], F32, tag="a2sb")
                    nc.vector.tensor_copy(a2sb[:], pa2[:])
                    nc.sync.dma_start(out=bufC[rows, WC - 1:WC],
                                      in_=a2sb[:, 0:1])
                    nc.sync.dma_start(out=a2dst[rows, :], in_=a2sb[:, 1:2])

                    # SAGE1
                    meanC = tl.tile([128, D], F32, tag="meanC")
                    nc.vector.tensor_copy(meanC[:], p_all[:, D:2 * D])
                    pmT = ptmp.tile([D, 128], F32, tag="pt")
                    nc.tensor.matmul(out=pmT[:], lhsT=meanC[:], rhs=ident[:],
                                     is_transpose=True)
                    meanT = tl.tile([D, 128], F32, tag="meanT")
                    nc.vector.tensor_copy(meanT[:], pmT[:])
                    xd0 = tl.tile([128, D], F16, tag="xd0")
                    nc.sync.dma_start(out=xd0[:], in_=dr["xs65"][rows, 0:D])
                    xd = tl.tile([128, D], F32, tag="xd")
                    nc.vector.tensor_copy(xd[:], xd0[:])
                    pxdT = ptmp.tile([D, 128], F32, tag="pt")
                    nc.tensor.matmul(out=pxdT[:], lhsT=xd[:], rhs=ident[:],
                                     is_transpose=True)
                    xdT = tl.tile([D, 128], F32, tag="xdT")
                    nc.vector.tensor_copy(xdT[:], pxdT[:])
                    psT = ptmp.tile([D, 128], F32, tag="pt")
                    nc.tensor.matmul(out=psT[:], lhsT=W["sage_wl1"][:],
                                     rhs=meanT[:], start=True, stop=False)
                    nc.tensor.matmul(out=psT[:], lhsT=W["sage_wr1"][:],
                                     rhs=xdT[:], start=False, stop=True)
                    sTs = tl.tile([D, 128], F32, tag="sTs")
                    nc.scalar.activation(out=sTs[:], in_=psT[:],
                                         func=AF.Identity,
                                         bias=W["sage_bl1"][:, :1])
                    sTv = tl.tile([D, 128], F32, tag="sTv")
                    nc.vector.tensor_copy(sTv[:], sTs[:])
                    ps_ = ptmp.tile([128, D], F32, tag="pt")
                    nc.tensor.matmul(out=ps_[:], lhsT=sTv[:], rhs=ident[:D, :D],
                                     is_transpose=True)
                    s_sb = tl.tile([128, D], F32, tag="s_sb")
                    nc.vector.tensor_copy(s_sb[:], ps_[:])
                    sq = tl.tile([128, D], F32, tag="sq")
                    nc.vector.tensor_tensor(out=sq[:], in0=s_sb[:], in1=s_sb[:],
                                            op=ALU.mult)
                    ssum = tl.tile([128, 1], F32, tag="ssum")
                    nc.vector.tensor_reduce(out=ssum[:], in_=sq[:],
                                            axis=mybir.AxisListType.X,
                                            op=ALU.add)
                    nc.vector.tensor_scalar(out=ssum[:], in0=ssum[:],
                                            scalar1=1e-24, scalar2=None,
                                            op0=ALU.add)
                    rs = tl.tile([128, 1], F32, tag="rs")
                    nc.vector.reciprocal(rs[:], ssum[:])
                    rq = tl.tile([128, 1], F32, tag="rq")
                    nc.scalar.activation(out=rq[:], in_=rs[:], func=AF.Sqrt)
                    nc.vector.tensor_scalar(out=st_hs[:, w * D:(w + 1) * D],
                                            in0=s_sb[:], scalar1=rq[:, :1],
                                            scalar2=0.0, op0=ALU.mult,
                                            op1=ALU.max)
                    nc.sync.dma_start(out=bufC[rows, 2 * D + 1:3 * D + 1],
                                      in_=st_hs[:, w * D:(w + 1) * D])

            # ============================================== AllGather #2
            nc.gpsimd.collective_compute(
                "AllGather", ALU.bypass,
                replica_groups=[list(range(NC_N))],
                ins=[bufC[:].opt()], outs=[gathC[:].opt()])

            # ================================================== phase C
            with (
                tc.tile_pool(name="gatC", bufs=8) as gat,
                tc.tile_pool(name="mC", bufs=8) as mpool,
                tc.tile_pool(name="tlC", bufs=4) as tl,
                tc.tile_pool(name="paccC", bufs=1, space="PSUM") as pacc,
                tc.tile_pool(name="ptmpC", bufs=2, space="PSUM") as ptmp,
            ):
                GW2 = 2 * D + D + 1             # 193
                t_glob = 0
                for w in range(nw):
                    ntw = tiles_w[w]
                    p_all = pacc.tile([128, GW2], F32, tag="p_all2")
                    for t in range(ntw):
                        Gt = gat.tile([128, WC], F32, tag="G2")
                        nc.gpsimd.indirect_dma_start(
                            out=Gt[:], out_offset=None, in_=gathC[:],
                            in_offset=bass.IndirectOffsetOnAxis(
                                ap=S["idx_row"][:, t_glob:t_glob + 1], axis=0))
                        sbt = gat.tile([128, 1], F32, tag="sb2")
                        nc.gpsimd.indirect_dma_start(
                            out=sbt[:], out_offset=None, in_=a2dst[:],
                            in_offset=bass.IndirectOffsetOnAxis(
                                ap=S["idx_colL"][:, t_glob:t_glob + 1], axis=0))
                        z2 = gat.tile([128, 1], F32, tag="z2")
                        nc.vector.tensor_tensor(
                            out=z2[:], in0=Gt[:, WC - 1:WC], in1=sbt[:],
                            op=ALU.add)
                        z2s = gat.tile([128, 1], F32, tag="z2s")
                        nc.vector.tensor_scalar(out=z2s[:], in0=z2[:],
                                                scalar1=NEG_SLOPE, scalar2=None,
                                                op0=ALU.mult)
                        nc.vector.tensor_tensor(out=z2[:], in0=z2[:], in1=z2s[:],
                                                op=ALU.max)
                        e2 = gat.tile([128, 1], F32, tag="E2")
                        nc.scalar.activation(out=e2[:], in_=z2[:], func=AF.Exp)

                        cr = S["colrel"][:, t_glob:t_glob + 1]
                        st, sp = (t == 0), (t == ntw - 1)
                        M0 = mpool.tile([128, 128], F32, tag="M0")
                        nc.vector.tensor_scalar(
                            out=M0[:], in0=iota_f[:], scalar1=cr,
                            scalar2=None, op0=ALU.is_equal)
                        GA = mpool.tile([128, GW2], F32, tag="GA2")
                        nc.vector.tensor_scalar(
                            out=GA[:, 0:D], in0=Gt[:, 0:D],
                            scalar1=S["wnorm"][:, t_glob:t_glob + 1],
                            scalar2=None, op0=ALU.mult)
                        nc.vector.tensor_scalar(
                            out=GA[:, D:2 * D], in0=Gt[:, 2 * D + 1:3 * D + 1],
                            scalar1=S["wsage"][:, t_glob:t_glob + 1],
                            scalar2=None, op0=ALU.mult)
                        nc.vector.tensor_scalar(
                            out=GA[:, 2 * D:GW2], in0=Gt[:, D:2 * D + 1],
                            scalar1=e2[:, 0:1],
                            scalar2=None, op0=ALU.mult)
                        nc.tensor.matmul(out=p_all[:], lhsT=M0[:], rhs=GA[:],
                                         start=st, stop=sp)
                        t_glob += 1

                    # ---------- window tails ----------
                    # GCN2 (+w0, +w0*b2)
                    aggC = tl.tile([128, D], F32, tag="aggC")
                    nc.vector.tensor_copy(aggC[:], p_all[:, 0:D])
                    paT = ptmp.tile([D, 128], F32, tag="pt")
                    nc.tensor.matmul(out=paT[:], lhsT=aggC[:], rhs=ident[:],
                                     is_transpose=True)
                    aggT = tl.tile([D, 128], F32, tag="aggT")
                    nc.vector.tensor_copy(aggT[:], paT[:])
                    poT = ptmp.tile([D, 128], F32, tag="pt")
                    nc.tensor.matmul(out=poT[:], lhsT=W["gcn_w2"][:],
                                     rhs=aggT[:])
                    oTs = tl.tile([D, 128], F32, tag="oTs")
                    nc.scalar.activation(out=oTs[:], in_=poT[:],
                                         func=AF.Identity,
                                         scale=w64[:, 0:1], bias=b2w0[:, :1])
                    oTv = tl.tile([D, 128], F32, tag="oTv")
                    nc.vector.tensor_copy(oTv[:], oTs[:])
                    po = ptmp.tile([128, D], F32, tag="pt")
                    nc.tensor.matmul(out=po[:], lhsT=oTv[:], rhs=ident[:D, :D],
                                     is_transpose=True)
                    ogcn = tl.tile([128, D], F32, tag="ogcn")
                    nc.vector.tensor_copy(ogcn[:], po[:])

                    # GAT2 (+w1)
                    rd = tl.tile([128, 1], F32, tag="rd")
                    nc.vector.reciprocal(rd[:], p_all[:, 3 * D:3 * D + 1])
                    ogat = tl.tile([128, D], F32, tag="ogat")
                    nc.vector.tensor_scalar(out=ogat[:],
                                            in0=p_all[:, 2 * D:3 * D],
                                            scalar1=rd[:, :1],
                                            scalar2=wc[:, 1:2],
                                            op0=ALU.mult, op1=ALU.mult)

                    # SAGE2 (+w2); root rows come from the SBUF staging
                    meanC = tl.tile([128, D], F32, tag="meanC")
                    nc.vector.tensor_copy(meanC[:], p_all[:, D:2 * D])
                    pmT = ptmp.tile([D, 128], F32, tag="pt")
                    nc.tensor.matmul(out=pmT[:], lhsT=meanC[:], rhs=ident[:],
                                     is_transpose=True)
                    meanT = tl.tile([D, 128], F32, tag="meanT")
                    nc.vector.tensor_copy(meanT[:], pmT[:])
                    phdT = ptmp.tile([D, 128], F32, tag="pt")
                    nc.tensor.matmul(out=phdT[:],
                                     lhsT=st_hs[:, w * D:(w + 1) * D],
                                     rhs=ident[:], is_transpose=True)
                    hdT = tl.tile([D, 128], F32, tag="hdT")
                    nc.vector.tensor_copy(hdT[:], phdT[:])
                    psT = ptmp.tile([D, 128], F32, tag="pt")
                    nc.tensor.matmul(out=psT[:], lhsT=W["sage_wl2"][:],
                                     rhs=meanT[:], start=True, stop=False)
                    nc.tensor.matmul(out=psT[:], lhsT=W["sage_wr2"][:],
                                     rhs=hdT[:], start=False, stop=True)
                    sTs = tl.tile([D, 128], F32, tag="sTs")
                    nc.scalar.activation(out=sTs[:], in_=psT[:],
                                         func=AF.Identity,
                                         bias=W["sage_bl2c"][:, :1])
                    sTv = tl.tile([D, 128], F32, tag="sTv")
                    nc.vector.tensor_copy(sTv[:], sTs[:])
                    ps_ = ptmp.tile([128, D], F32, tag="pt")
                    nc.tensor.matmul(out=ps_[:], lhsT=sTv[:], rhs=ident[:D, :D],
                                     is_transpose=True)
                    s_sb = tl.tile([128, D], F32, tag="s_sb")
                    nc.vector.tensor_copy(s_sb[:], ps_[:])
                    sq = tl.tile([128, D], F32, tag="sq")
                    nc.vector.tensor_tensor(out=sq[:], in0=s_sb[:], in1=s_sb[:],
                                            op=ALU.mult)
                    ssum = tl.tile([128, 1], F32, tag="ssum")
                    nc.vector.tensor_reduce(out=ssum[:], in_=sq[:],
                                            axis=mybir.AxisListType.X,
                                            op=ALU.add)
                    nc.vector.tensor_scalar(out=ssum[:], in0=ssum[:],
                                            scalar1=1e-24, scalar2=None,
                                            op0=ALU.add)
                    rs = tl.tile([128, 1], F32, tag="rs")
                    nc.vector.reciprocal(rs[:], ssum[:])
                    rq = tl.tile([128, 1], F32, tag="rq")
                    nc.scalar.activation(out=rq[:], in_=rs[:], func=AF.Sqrt)
                    osage = tl.tile([128, D], F32, tag="osage")
                    nc.vector.tensor_scalar(out=osage[:], in0=s_sb[:],
                                            scalar1=rq[:, :1],
                                            scalar2=wc[:, 2:3],
                                            op0=ALU.mult, op1=ALU.mult)

                    # mix
                    mx1 = tl.tile([128, D], F32, tag="mx1")
                    nc.vector.tensor_tensor(out=mx1[:], in0=ogcn[:],
                                            in1=ogat[:], op=ALU.add)
                    mx2 = tl.tile([128, D], F32, tag="mx2")
                    nc.vector.tensor_tensor(out=mx2[:], in0=mx1[:],
                                            in1=osage[:], op=ALU.add)
                    nc.vector.tensor_tensor(out=st_out[:, w * D:(w + 1) * D],
                                            in0=mx2[:], in1=bgat[:],
                                            op=ALU.add)

            # ---- final output DMA: full windows, then the partial tail
            out_ap = bass.AP(out, 0, [[D, 128], [128 * D, nw - 1], [1, D]])
            nc.sync.dma_start(
                out=out_ap,
                in_=st_out[:, 0:(nw - 1) * D].rearrange(
                    "p (w c) -> p w c", w=nw - 1))
            nc.sync.dma_start(
                out=out[(nw - 1) * 128:shard, :],
                in_=st_out[0:rem, (nw - 1) * D:nw * D])
    return nc


# ---------------------------------------------------------------- host logic
DEBUG = {}
_PROG_CACHE = {}


def _run(nc, in_maps, trace=False):
    import time as _time
    if not nc.is_finalized():
        nc.finalize()
        # the jitted wrapper re-serializes the (frozen) module on every
        # call (~0.3s for this program); memoize on the instance
        _bir = nc.to_json_bytes()
        nc.to_json_bytes = lambda _b=_bir: _b
    t0 = _time.perf_counter()
    res = run_bass_kernel_spmd(nc, in_maps, list(range(NC_N)), trace=trace)
    DEBUG.setdefault("run_walls", []).append(_time.perf_counter() - t0)
    if res.exec_time_ns:
        DEBUG.setdefault("exec_ns", []).append(res.exec_time_ns)
    return res.results


def gnn_forward(x, edge_index, gate_w1, gate_b1, gate_w2, gate_b2,
                gcn_w1, gcn_b1, bn_gamma, bn_beta, gcn_w2, gcn_b2,
                gat_w1, gat_att_src1, gat_att_dst1, gat_b1,
                gat_w2, gat_att_src2, gat_att_dst2, gat_b2,
                sage_wl1, sage_bl1, sage_wr1, sage_wl2, sage_bl2, sage_wr2,
                trace=False):
    n_nodes = x.shape[0]
    x = np.asarray(x, np.float32)
    streams, tiles_w, T, shard, nw = build_schedule(
        np.asarray(edge_index), n_nodes)
    npad = nw * 128

    # ---- host weight folding (weights only, no data)
    w1r = np.asarray(gat_w1, np.float32).reshape(D, H1, D)
    vsrc = np.einsum("chj,hj->ch", w1r, np.asarray(gat_att_src1, np.float32))
    vdst = np.einsum("chj,hj->ch", w1r, np.asarray(gat_att_dst1, np.float32))
    vcat = np.concatenate([vsrc, vdst], axis=1).astype(np.float32)  # [64,8]
    v2 = (np.asarray(gat_w2, np.float32) @
          np.asarray(gat_att_src2, np.float32)[0])  # [256]
    u2 = (np.asarray(gat_w2, np.float32) @
          np.asarray(gat_att_dst2, np.float32)[0])
    v2u2 = np.stack([v2[:128], u2[:128], v2[128:], u2[128:]],
                    axis=1).astype(np.float32)  # [128,4]
    bn_s = (np.asarray(bn_gamma, np.float32) /
            np.sqrt(np.float32(1.0 + BN_EPS)))
    gcn1_s = bn_s.reshape(D, 1).astype(np.float32)
    gcn1_b = (bn_s * np.asarray(gcn_b1, np.float32) +
              np.asarray(bn_beta, np.float32)).reshape(D, 1).astype(np.float32)

    ck = (n_nodes, T, tuple(tiles_w))
    if ck in _PROG_CACHE:
        nc = _PROG_CACHE[ck]
    else:
        nc = build_full(n_nodes, tiles_w, T)
        _PROG_CACHE[ck] = nc

    common = {
        "vcat": vcat,
        "gw1": np.asarray(gate_w1, np.float32),
        "gb1": np.asarray(gate_b1, np.float32).reshape(1, D),
        "gw2": np.asarray(gate_w2, np.float32),
        "gb2": np.asarray(gate_b2, np.float32).reshape(1, 3),
        "gcn_w1": np.asarray(gcn_w1, np.float32),
        "gcn1_s": gcn1_s, "gcn1_b": gcn1_b,
        "sage_wl1": np.asarray(sage_wl1, np.float32),
        "sage_wr1": np.asarray(sage_wr1, np.float32),
        "sage_bl1": np.asarray(sage_bl1, np.float32).reshape(D, 1),
        "w2A": np.asarray(gat_w2, np.float32)[:128],
        "w2B": np.asarray(gat_w2, np.float32)[128:],
        "v2u2": v2u2,
        "w1h": np.asarray(gat_w1, np.float32),
        "b1c": np.asarray(gat_b1, np.float32).reshape(2, 128).T.copy(),
        "gcn_w2": np.asarray(gcn_w2, np.float32),
        "gcn_b2c": np.asarray(gcn_b2, np.float32).reshape(D, 1),
        "sage_wl2": np.asarray(sage_wl2, np.float32),
        "sage_wr2": np.asarray(sage_wr2, np.float32),
        "sage_bl2c": np.asarray(sage_bl2, np.float32).reshape(D, 1),
        "gat_b2r": np.asarray(gat_b2, np.float32).reshape(1, D),
    }
    in_maps = []
    for k in range(NC_N):
        m = dict(common)
        m.update(streams[k])
        xs65 = np.zeros((npad, D + 1), np.float16)
        xs65[:shard, :D] = x[k * shard:(k + 1) * shard]
        xs65[:shard, D] = 1.0
        m["xs65"] = xs65
        in_maps.append(m)
    res = _run(nc, in_maps, trace=trace)
    out = np.concatenate([res[k]["out"] for k in range(NC_N)], 0)
    return out.astype(np.float32)


def kernel(**inputs):
    return gnn_forward(**inputs)
